# revision 1
# baseline (speedup 1.0000x reference)
"""Two-layer RGAT (R=3, heads=1) on 8 trn2 NeuronCores.

Strategy (dst-sharded, one-hot-matmul aggregation):
  - Nodes padded to 50176 = 8 cores x 49 blocks x 128; core c owns dst nodes
    [c*6272, (c+1)*6272) and computes the full output rows for them.
  - Per layer, each core computes its slice of the per-relation node transform
    xw[r] = x @ W_r (plus attention scalars ak = xw@k, aq = xw@q) into a DRAM
    table (row = (src_core, rt, src_local), 192-f32 stride, 130 payload:
    [128 feats | 1.0 | ak]); AllGather replicates the table.
  - Edges (sorted by dst block, then by table-row range so int16 gather
    indices fit) are processed in 128-edge chunks: dma_gather fetches the
    chunk's source rows; alpha = exp(LeakyRelu(aq[rt,dst] + ak[rt,src] +
    c_l*ea)) is built from a second (local) aq-table gather; a fused DVE
    tensor_scalar builds the alpha-scaled one-hot O[e, dst_local]; one
    matmul per chunk accumulates psum[node,129] = [sum alpha*xj | sum alpha].
  - Block results accumulate in SBUF across range-phases; finalize divides by
    the denominator, adds bias (+ReLU for layer 1). Layer-2 output rows DMA
    straight to the per-core output; the host concatenates and trims.
"""
import sys
sys.path.insert(0, '/opt/trn_rl_repo')
import inspect
import textwrap
import numpy as np

import concourse.bass as bass
import concourse.bacc as bacc
import concourse.mybir as mybir
from concourse.bass_utils import run_bass_kernel_spmd
from concourse.tile import TileContext
from concourse.masks import make_identity

F32 = mybir.dt.float32
I16 = mybir.dt.int16
I32 = mybir.dt.int32
NEG_SLOPE = 0.2

# ---- relax dma_gather's elem_size%256 restriction (descriptor length is ----
# ---- arbitrary; only the row *stride* must be a multiple of 256B)       ----
_src = inspect.getsource(bass.BassGpSimd.dma_gather)
_src = _src.replace(
    "elem_size_bytes > 0 and elem_size_bytes % 256 == 0",
    "elem_size_bytes > 0",
)
_ns = {}
exec(compile(textwrap.dedent(_src), "<dma_gather_patched>", "exec"), dict(vars(bass)), _ns)
bass.BassGpSimd.dma_gather = _ns["dma_gather"]


class Cfg:
    pass


def make_cfg(N, E, NC=8, GCALL=32, RANGE=32768):
    cfg = Cfg()
    cfg.NC = NC
    cfg.N, cfg.E = N, E
    cfg.NPAD = -(-N // (128 * NC)) * 128 * NC
    cfg.NPC = cfg.NPAD // NC
    cfg.NBLK = cfg.NPC // 128
    cfg.RPC = 3 * cfg.NPC
    cfg.RTOT = cfg.RPC * NC
    cfg.RANGE = RANGE
    cfg.NPH = -(-cfg.RTOT // RANGE)
    cfg.GCALL = GCALL
    return cfg


def host_prep(cfg, x, edge_index, edge_type, edge_attr, w1, q1, k1, le1, e1, b1,
              w2, q2, k2, le2, e2, b2):
    """Returns (per_core_inputs list, cfg with CP/calls/NCH set)."""
    NC, NPC, NBLK, RANGE = cfg.NC, cfg.NPC, cfg.NBLK, cfg.RANGE
    src, dst = edge_index[0].astype(np.int64), edge_index[1].astype(np.int64)
    rt = edge_type.astype(np.int64)
    ea = edge_attr[:, 0].astype(np.float32)
    c1 = float(le1.reshape(-1) @ e1.reshape(-1))
    c2 = float(le2.reshape(-1) @ e2.reshape(-1))

    core = dst // NPC
    blk = (dst % NPC) // 128
    dl = dst % 128
    grow = (src // NPC) * cfg.RPC + rt * NPC + (src % NPC)
    ph = grow // RANGE
    lidx = grow - ph * RANGE
    aqi = rt * NPC + (dst % NPC)

    # per (core, blk, phase) counts -> CPB[p][b] = max-over-cores chunks
    counts = np.zeros((NC, NBLK, cfg.NPH), np.int64)
    np.add.at(counts, (core, blk, ph), 1)
    CPB = -(-counts.max(axis=0) // 128)          # [NBLK, NPH]
    cfg.CPB = CPB
    # slot layout: phase-major; within phase, blocks at cumsum offsets
    cfg.pboff = np.zeros((cfg.NPH, NBLK), np.int64)
    base = [0]
    for p in range(cfg.NPH):
        cfg.pboff[p] = np.concatenate([[0], np.cumsum(CPB[:-1, p])])
        base.append(base[-1] + int(CPB[:, p].sum()))
    cfg.base = np.asarray(base, np.int64)
    cfg.NCH = int(cfg.base[-1])

    # gather call list: per phase, contiguous slot runs of <= GCALL slots
    calls = []
    for p in range(cfg.NPH):
        nslots = int(CPB[:, p].sum())
        s = 0
        while s < nslots:
            ns = min(cfg.GCALL, nslots - s)
            calls.append((p, int(cfg.base[p] + s), int(ns)))
            s += ns
    cfg.calls = calls

    def pack16(vals):
        """vals [NCH*128] -> packed idx tile [128, NCH*8], per-call layout."""
        out = np.zeros((128, cfg.NCH * 8), np.int16)
        for (p, s0, ns) in calls:
            v = vals[s0 * 128:(s0 + ns) * 128]
            i = np.arange(ns * 128)
            cols = s0 * 8 + i // 16
            rows = i % 16
            for g in range(8):
                out[rows + 16 * g, cols] = v
        return out

    # weight packs
    def wpack(w, qv, kv):
        W = np.zeros((128, 393), np.float32)
        for r in range(3):
            W[:, r * 130:r * 130 + 128] = w[r]
            W[:, r * 130 + 129] = (w[r] @ kv).ravel()
            W[:, 390 + r] = (w[r] @ qv).ravel()
        return W

    W1p, W2p = wpack(w1, q1, k1), wpack(w2, q2, k2)

    per_core = []
    for c in range(NC):
        m = core == c
        eb, ep = blk[m], ph[m]
        edl, elx, eaq = dl[m], lidx[m], aqi[m]
        eea = ea[m]
        order = np.lexsort((ep, eb))
        eb, ep, edl, elx, eaq, eea = (a[order] for a in (eb, ep, edl, elx, eaq, eea))
        # rank within (blk, phase) group
        gid = eb * cfg.NPH + ep
        sortg = np.argsort(gid, kind='stable')
        assert (sortg == np.arange(len(gid))).all()  # already sorted
        boundaries = np.concatenate([[0], np.cumsum(np.bincount(gid.astype(np.int64),
                                                                minlength=NBLK * cfg.NPH))])
        rank = np.arange(len(gid)) - boundaries[gid]
        slot = cfg.base[ep] + cfg.pboff[ep, eb] + rank // 128
        prow = rank % 128

        dst_s = np.full((128, cfg.NCH), -1.0, np.float32)
        et_s = np.zeros((128, 2 * cfg.NCH), np.float32)
        fidx_v = np.zeros(cfg.NCH * 128, np.int64)
        aq_v = np.zeros(cfg.NCH * 128, np.int64)
        dst_s[prow, slot] = edl
        et_s[prow, slot] = c1 * eea
        et_s[prow, cfg.NCH + slot] = c2 * eea
        fidx_v[slot * 128 + prow] = elx
        aq_v[slot * 128 + prow] = eaq

        xs = np.zeros((cfg.NPC, x.shape[1]), np.float32)
        lo, hi = c * NPC, min((c + 1) * NPC, cfg.N)
        if hi > lo:
            xs[:hi - lo] = x[lo:hi]
        per_core.append({
            "xT": np.ascontiguousarray(xs.T),
            "W1": W1p, "W2": W2p,
            "B1": b1.reshape(1, 128).astype(np.float32),
            "B2": b2.reshape(1, 128).astype(np.float32),
            "DSTS": dst_s, "ET": et_s,
            "FIDX": pack16(fidx_v), "AQIX": pack16(aq_v),
        })
    return per_core


def build_nc(cfg, skips=()):
    skips = set(skips)
    nc = bacc.Bacc("TRN2", target_bir_lowering=False, num_swdge_queues=4)
    NPC, NBLK, NCH = cfg.NPC, cfg.NBLK, cfg.NCH

    xT = nc.declare_dram_parameter("xT", [128, NPC], F32, isOutput=False)
    W = {1: nc.declare_dram_parameter("W1", [128, 393], F32, isOutput=False),
         2: nc.declare_dram_parameter("W2", [128, 393], F32, isOutput=False)}
    B = {1: nc.declare_dram_parameter("B1", [1, 128], F32, isOutput=False),
         2: nc.declare_dram_parameter("B2", [1, 128], F32, isOutput=False)}
    DSTS = nc.declare_dram_parameter("DSTS", [128, NCH], F32, isOutput=False)
    ET = nc.declare_dram_parameter("ET", [128, 2 * NCH], F32, isOutput=False)
    FIDX = nc.declare_dram_parameter("FIDX", [128, NCH * 8], I16, isOutput=False)
    AQIX = nc.declare_dram_parameter("AQIX", [128, NCH * 8], I16, isOutput=False)
    OUT2 = nc.declare_dram_parameter("out2", [NPC, 128], F32, isOutput=True)

    tabs = {L: nc.dram_tensor(f"tabs{L}", [cfg.RPC, 192], F32) for L in (1, 2)}
    tabg = {L: nc.dram_tensor(f"tabg{L}", [cfg.RTOT, 192], F32, addr_space="Shared")
            for L in (1, 2)}
    aqt = {L: nc.dram_tensor(f"aqt{L}", [cfg.RPC, 64], F32) for L in (1, 2)}

    AL = mybir.AluOpType
    AF = mybir.ActivationFunctionType

    with TileContext(nc) as tc:
        with (
            tc.tile_pool(name="const", bufs=1) as cp,
            tc.tile_pool(name="stag", bufs=4) as sp,
            tc.tile_pool(name="aqs", bufs=6) as qp,
            tc.tile_pool(name="oa", bufs=8) as op,
            tc.tile_pool(name="work", bufs=3) as wp,
            tc.tile_pool(name="pacc", bufs=4, space="PSUM") as pa,
            tc.tile_pool(name="ptab", bufs=2, space="PSUM") as pt,
            tc.tile_pool(name="pmisc", bufs=2, space="PSUM") as px,
        ):
            # ---- constants / staged inputs ----
            xT_t = cp.tile([128, NPC], F32)
            nc.sync.dma_start(out=xT_t[:], in_=xT[:])
            W_t = {L: cp.tile([128, 393], F32, tag=f"W{L}", name=f"W{L}_t") for L in (1, 2)}
            B_t = {L: cp.tile([1, 128], F32, tag=f"B{L}", name=f"B{L}_t") for L in (1, 2)}
            for L in (1, 2):
                nc.sync.dma_start(out=W_t[L][:], in_=W[L][:])
                nc.sync.dma_start(out=B_t[L][:], in_=B[L][:])
            dst_t = cp.tile([128, NCH], F32)
            nc.sync.dma_start(out=dst_t[:], in_=DSTS[:])
            et_t = cp.tile([128, 2 * NCH], F32)
            nc.sync.dma_start(out=et_t[:], in_=ET[:])
            fidx_t = cp.tile([128, NCH * 8], I16)
            nc.sync.dma_start(out=fidx_t[:], in_=FIDX[:])
            aqix_t = cp.tile([128, NCH * 8], I16)
            nc.sync.dma_start(out=aqix_t[:], in_=AQIX[:])

            ii = cp.tile([128, 128], I32)
            nc.gpsimd.iota(ii[:], pattern=[[1, 128]], base=0, channel_multiplier=0)
            iof = cp.tile([128, 128], F32)
            nc.vector.tensor_copy(iof[:], ii[:])
            ident = cp.tile([128, 128], F32)
            make_identity(nc, ident[:])
            ones1 = cp.tile([1, 128], F32)
            nc.vector.memset(ones1[:], 1.0)

            out_sb = cp.tile([128, NBLK * 129], F32)
            h_all = cp.tile([128, NBLK * 128], F32)
            aq_all = cp.tile([128, 3 * NBLK], F32)
            bias_bc = cp.tile([128, 128], F32)

            qrr = [0]

            def qn():
                qrr[0] = (qrr[0] + 1) % 4
                return qrr[0]

            for L in (1, 2):
                # ---- bias broadcast [128,128] ----
                pb = px.tile([128, 128], F32, tag="pmisc")
                nc.tensor.matmul(pb[:], lhsT=ones1[:], rhs=B_t[L][:], start=True, stop=True)
                nc.vector.tensor_copy(bias_bc[:], pb[:])

                # ---- node transform table build ----
                for t in range(NBLK):
                    if L == 1:
                        lhs = xT_t[:, t * 128:(t + 1) * 128]
                    else:
                        pT = px.tile([128, 128], F32, tag="pmisc")
                        nc.tensor.transpose(pT[:], h_all[:, t * 128:(t + 1) * 128], ident[:])
                        hT = wp.tile([128, 128], F32, tag="hT")
                        nc.vector.tensor_copy(hT[:], pT[:])
                        lhs = hT[:]
                    ptab = pt.tile([128, 393], F32)
                    nc.tensor.matmul(ptab[:], lhsT=lhs, rhs=W_t[L][:], start=True, stop=True)
                    stab = wp.tile([128, 390], F32, tag="stab")
                    nc.vector.tensor_copy(stab[:], ptab[:, 0:390])
                    for r in range(3):
                        nc.vector.memset(stab[:, r * 130 + 128:r * 130 + 129], 1.0)
                        nc.vector.tensor_copy(aq_all[:, r * NBLK + t:r * NBLK + t + 1],
                                              ptab[:, 390 + r:391 + r])
                    for r in range(3):
                        nc.sync.dma_start(
                            out=tabs[L][r * NPC + t * 128:r * NPC + (t + 1) * 128, 0:130],
                            in_=stab[:, r * 130:r * 130 + 130])
                for r in range(3):
                    dstv = aqt[L][r * NPC:(r + 1) * NPC, 0:1] \
                        .rearrange("(t p) o -> p (t o)", p=128)
                    nc.sync.dma_start(out=dstv, in_=aq_all[:, r * NBLK:(r + 1) * NBLK])

                # ---- AllGather the table ----
                nc.gpsimd.collective_compute(
                    "AllGather", AL.bypass, replica_groups=[list(range(cfg.NC))],
                    ins=[tabs[L][:]], outs=[tabg[L][:]])

                # ---- main edge loop ----
                nc.vector.memset(out_sb[:], 0.0)
                loff = (L - 1) * NCH
                call_tiles = {}
                expa_tiles = {}
                for (p, s0, ns) in cfg.calls:
                    vrows = min(cfg.RANGE, cfg.RTOT - p * cfg.RANGE)
                    fst = sp.tile([128, cfg.GCALL, 130], F32, tag="fst")
                    if 'gather' in skips:
                        nc.vector.memset(fst[:, 0, 0:2], 0.0)
                    else: nc.gpsimd.dma_gather(
                        fst[:, :ns, :],
                        tabg[L][p * cfg.RANGE:p * cfg.RANGE + vrows, 0:130],
                        fidx_t[:, s0 * 8:(s0 + ns) * 8],
                        ns * 128, ns * 128, 130, elem_step=192,
                        single_packet=False, queue_num=qn())
                    aqs = qp.tile([128, cfg.GCALL, 1], F32, tag="aqs")
                    if 'aq' in skips:
                        nc.vector.memset(aqs[:, 0, 0:1], 0.0)
                    else: nc.gpsimd.dma_gather(
                        aqs[:, :ns, :], aqt[L][:, 0:1],
                        aqix_t[:, s0 * 8:(s0 + ns) * 8],
                        ns * 128, ns * 128, 1, elem_step=64,
                        single_packet=False, queue_num=qn())
                    ext = qp.tile([128, cfg.GCALL], F32, tag="ext")
                    sl = ext[:, :ns]
                    if 'alpha' in skips:
                        nc.vector.memset(ext[:, 0:2], 0.0)
                    if 'alpha' not in skips:
                        nc.vector.tensor_tensor(sl, aqs[:, :ns, 0], fst[:, :ns, 129], op=AL.add)
                        nc.vector.tensor_tensor(sl, sl, et_t[:, loff + s0:loff + s0 + ns], op=AL.add)
                        lrt = wp.tile([128, cfg.GCALL], F32, tag="lrt")
                        nc.vector.tensor_scalar_mul(lrt[:, :ns], sl, NEG_SLOPE)
                        nc.vector.tensor_tensor(sl, sl, lrt[:, :ns], op=AL.max)
                        nc.scalar.activation(sl, sl, AF.Exp)
                    for k in range(ns):
                        call_tiles[s0 + k] = (fst, k)
                        expa_tiles[s0 + k] = (ext, k)

                for grp in [(p,) for p in range(cfg.NPH)]:
                    for b in range(NBLK):
                        slots = [int(cfg.base[p] + cfg.pboff[p, b] + c)
                                 for p in grp for c in range(int(cfg.CPB[b, p]))]
                        if not slots:
                            continue
                        pacc = pa.tile([128, 129], F32)
                        if 'mm' in skips:
                            nc.vector.memset(pacc[:, 0:2], 0.0)
                        for ci, s in enumerate(slots):
                            fst, ls = call_tiles[s]
                            oa = op.tile([128, 128], F32, tag="oa")
                            ext, ek = expa_tiles[s]
                            if 'oa' in skips:
                                nc.vector.memset(oa[:, 0:2], 0.0)
                            if 'oa' not in skips:
                                nc.vector.tensor_scalar(
                                    oa[:], iof[:], dst_t[:, s:s + 1], ext[:, ek:ek + 1],
                                    op0=AL.is_equal, op1=AL.mult)
                            if 'mm' not in skips:
                                nc.tensor.matmul(pacc[:], lhsT=oa[:], rhs=fst[:, ls, 0:129],
                                                 start=(ci == 0), stop=(ci == len(slots) - 1))
                        if 'evac' not in skips:
                            nc.vector.tensor_tensor(out_sb[:, b * 129:(b + 1) * 129],
                                                    out_sb[:, b * 129:(b + 1) * 129],
                                                    pacc[:], op=AL.add)

                # ---- finalize ----
                for b in range(NBLK):
                    rc = wp.tile([128, 1], F32, tag="rc")
                    nc.vector.tensor_scalar_add(rc[:], out_sb[:, b * 129 + 128:b * 129 + 129],
                                                1e-16)
                    nc.vector.reciprocal(rc[:], rc[:])
                    if L == 1:
                        tgt = h_all[:, b * 128:(b + 1) * 128]
                    else:
                        ot = wp.tile([128, 128], F32, tag="ot")
                        tgt = ot[:]
                    nc.vector.tensor_scalar_mul(tgt, out_sb[:, b * 129:b * 129 + 128], rc[:])
                    nc.vector.tensor_tensor(tgt, tgt, bias_bc[:], op=AL.add)
                    if L == 1:
                        nc.vector.tensor_scalar_max(tgt, tgt, 0.0)
                    else:
                        nc.sync.dma_start(out=OUT2[b * 128:(b + 1) * 128, :], in_=tgt)
    nc.compile()
    return nc


_CACHE = {}


def run(x, edge_index, edge_type, edge_attr, w1, q1, k1, le1, e1, b1,
        w2, q2, k2, le2, e2, b2, N=None, E=None):
    x = np.asarray(x, np.float32)
    N = x.shape[0] if N is None else N
    E = edge_index.shape[1] if E is None else E
    cfg = make_cfg(N, E)
    per_core = host_prep(cfg, x, np.asarray(edge_index), np.asarray(edge_type),
                         np.asarray(edge_attr, np.float32),
                         np.asarray(w1, np.float32), np.asarray(q1, np.float32),
                         np.asarray(k1, np.float32), np.asarray(le1, np.float32),
                         np.asarray(e1, np.float32), np.asarray(b1, np.float32),
                         np.asarray(w2, np.float32), np.asarray(q2, np.float32),
                         np.asarray(k2, np.float32), np.asarray(le2, np.float32),
                         np.asarray(e2, np.float32), np.asarray(b2, np.float32))
    key = (N, E, cfg.NCH, cfg.CPB.sum())
    if key not in _CACHE:
        _CACHE[key] = build_nc(cfg)
    nc = _CACHE[key]
    res = run_bass_kernel_spmd(nc, per_core, core_ids=list(range(cfg.NC)))
    out = np.concatenate([res.results[c]["out2"] for c in range(cfg.NC)], axis=0)
    return out[:N]


def kernel(**inputs):
    return run(
        inputs["x"], inputs["edge_index"], inputs["edge_type"], inputs["edge_attr"],
        inputs["w1"], inputs["q1"], inputs["k1"], inputs["le1"], inputs["e1"], inputs["b1"],
        inputs["w2"], inputs["q2"], inputs["k2"], inputs["le2"], inputs["e2"], inputs["b2"],
    ).astype(np.float32)



# revision 3
# speedup vs baseline: 1.6187x; 1.6187x over previous
"""Two-layer RGAT (R=3, heads=1) on 8 trn2 NeuronCores.

Strategy (dst-sharded, one-hot-matmul aggregation), I/O-lean variant:
  - Nodes padded to 50176 = 8 cores x 49 blocks x 128; core c owns dst nodes
    [c*6272, (c+1)*6272) and computes the full output rows for them.
  - Per layer, each core computes its slice of the per-relation node transform
    xw[r] = x @ W_r (plus attention scalars ak = xw@k, aq = xw@q) into a DRAM
    table (row = (src_core, rt, src_local), 256-f16 stride, 130 payload:
    [128 feats | 1.0 | ak]); AllGather replicates the table.
  - Edges (sorted by dst block, then by table-row range so int16 gather
    indices fit) are processed in 128-edge chunks: dma_gather fetches the
    chunk's source rows; alpha = exp(LeakyRelu(aq[rt,dst] + ak[rt,src] +
    c_l*ea)) is built from a second (local) aq-table gather; a fused DVE
    tensor_scalar builds the alpha-scaled one-hot O[e, dst_local]; one
    matmul per chunk accumulates psum[node,129] = [sum alpha*xj | sum alpha].
  - Block results accumulate in SBUF across range-phases; finalize divides by
    the denominator, adds bias (+ReLU for layer 1). Layer-2 output rows DMA
    straight to the per-core output; the host concatenates and trims.

  Host<->device traffic is the wall-clock bottleneck (axon tunnel), so all
  bulk tensors are f16 and index tiles ship unreplicated ([16, NCH*8]; the
  gpsimd gather wants them replicated across the 8 16-partition groups, which
  the kernel does on-device with 8 DMAs). Per-edge scalars ship once (ea as
  f16; c1*ea / c2*ea are formed on-device from a [1,2] constant input).
"""
import sys
sys.path.insert(0, '/opt/trn_rl_repo')
import inspect
import textwrap
import numpy as np

import concourse.bass as bass
import concourse.bacc as bacc
import concourse.mybir as mybir
from concourse.bass_utils import run_bass_kernel_spmd
from concourse.tile import TileContext
from concourse.masks import make_identity

F32 = mybir.dt.float32
F16 = mybir.dt.float16
I16 = mybir.dt.int16
I32 = mybir.dt.int32
NEG_SLOPE = 0.2

# ---- relax dma_gather's elem_size%256 restriction (descriptor length is ----
# ---- arbitrary; only the row *stride* must be a multiple of 256B)       ----
_src = inspect.getsource(bass.BassGpSimd.dma_gather)
_src = _src.replace(
    "elem_size_bytes > 0 and elem_size_bytes % 256 == 0",
    "elem_size_bytes > 0",
)
_ns = {}
exec(compile(textwrap.dedent(_src), "<dma_gather_patched>", "exec"), dict(vars(bass)), _ns)
bass.BassGpSimd.dma_gather = _ns["dma_gather"]


class Cfg:
    pass


def make_cfg(N, E, NC=8, GCALL=32, RANGE=32768):
    cfg = Cfg()
    cfg.NC = NC
    cfg.N, cfg.E = N, E
    cfg.NPAD = -(-N // (128 * NC)) * 128 * NC
    cfg.NPC = cfg.NPAD // NC
    cfg.NBLK = cfg.NPC // 128
    cfg.RPC = 3 * cfg.NPC
    cfg.RTOT = cfg.RPC * NC
    cfg.RANGE = RANGE
    cfg.NPH = -(-cfg.RTOT // RANGE)
    cfg.GCALL = GCALL
    return cfg


def host_prep(cfg, x, edge_index, edge_type, edge_attr, w1, q1, k1, le1, e1, b1,
              w2, q2, k2, le2, e2, b2):
    """Returns (per_core_inputs list, cfg with CP/calls/NCH set)."""
    NC, NPC, NBLK, RANGE = cfg.NC, cfg.NPC, cfg.NBLK, cfg.RANGE
    src, dst = edge_index[0].astype(np.int64), edge_index[1].astype(np.int64)
    rt = edge_type.astype(np.int64)
    ea = edge_attr[:, 0].astype(np.float32)
    c1 = float(le1.reshape(-1) @ e1.reshape(-1))
    c2 = float(le2.reshape(-1) @ e2.reshape(-1))

    core = dst // NPC
    blk = (dst % NPC) // 128
    dl = dst % 128
    grow = (src // NPC) * cfg.RPC + rt * NPC + (src % NPC)
    ph = grow // RANGE
    lidx = grow - ph * RANGE
    aqi = rt * NPC + (dst % NPC)

    # per (core, blk, phase) counts -> CPB[p][b] = max-over-cores chunks
    counts = np.zeros((NC, NBLK, cfg.NPH), np.int64)
    np.add.at(counts, (core, blk, ph), 1)
    CPB = -(-counts.max(axis=0) // 128)          # [NBLK, NPH]
    cfg.CPB = CPB
    # slot layout: phase-major; within phase, blocks at cumsum offsets
    cfg.pboff = np.zeros((cfg.NPH, NBLK), np.int64)
    base = [0]
    for p in range(cfg.NPH):
        cfg.pboff[p] = np.concatenate([[0], np.cumsum(CPB[:-1, p])])
        base.append(base[-1] + int(CPB[:, p].sum()))
    cfg.base = np.asarray(base, np.int64)
    cfg.NCH = int(cfg.base[-1])

    # gather call list: per phase, contiguous slot runs of <= GCALL slots
    calls = []
    for p in range(cfg.NPH):
        nslots = int(CPB[:, p].sum())
        s = 0
        while s < nslots:
            ns = min(cfg.GCALL, nslots - s)
            calls.append((p, int(cfg.base[p] + s), int(ns)))
            s += ns
    cfg.calls = calls

    def pack16(vals):
        """vals [NCH*128] -> unreplicated idx tile [16, NCH*8].

        dma_gather wants [128, NCH*8] with the 16-row pattern replicated
        across the 8 gpsimd core groups; the kernel replicates on-device.
        Layout: value for (slot s, lane l) sits at [l%16, 8*s + l//16].
        """
        v = vals.reshape(cfg.NCH, 128)          # [s, l]
        out = v.reshape(cfg.NCH, 8, 16)         # [s, l//16, l%16]
        return np.ascontiguousarray(
            out.transpose(2, 0, 1).reshape(16, cfg.NCH * 8)).astype(np.int16)

    # weight packs
    def wpack(w, qv, kv):
        W = np.zeros((128, 393), np.float32)
        for r in range(3):
            W[:, r * 130:r * 130 + 128] = w[r]
            W[:, r * 130 + 129] = (w[r] @ kv).ravel()
            W[:, 390 + r] = (w[r] @ qv).ravel()
        return W.astype(np.float16)

    W1p, W2p = wpack(w1, q1, k1), wpack(w2, q2, k2)

    per_core = []
    for c in range(NC):
        m = core == c
        eb, ep = blk[m], ph[m]
        edl, elx, eaq = dl[m], lidx[m], aqi[m]
        eea = ea[m]
        order = np.lexsort((ep, eb))
        eb, ep, edl, elx, eaq, eea = (a[order] for a in (eb, ep, edl, elx, eaq, eea))
        # rank within (blk, phase) group
        gid = eb * cfg.NPH + ep
        sortg = np.argsort(gid, kind='stable')
        assert (sortg == np.arange(len(gid))).all()  # already sorted
        boundaries = np.concatenate([[0], np.cumsum(np.bincount(gid.astype(np.int64),
                                                                minlength=NBLK * cfg.NPH))])
        rank = np.arange(len(gid)) - boundaries[gid]
        slot = cfg.base[ep] + cfg.pboff[ep, eb] + rank // 128
        prow = rank % 128

        dst_s = np.full((128, cfg.NCH), -1.0, np.float16)
        ea_s = np.zeros((128, cfg.NCH), np.float16)
        fidx_v = np.zeros(cfg.NCH * 128, np.int64)
        aq_v = np.zeros(cfg.NCH * 128, np.int64)
        dst_s[prow, slot] = edl
        ea_s[prow, slot] = eea
        fidx_v[slot * 128 + prow] = elx
        aq_v[slot * 128 + prow] = eaq

        xs = np.zeros((cfg.NPC, x.shape[1]), np.float32)
        lo, hi = c * NPC, min((c + 1) * NPC, cfg.N)
        if hi > lo:
            xs[:hi - lo] = x[lo:hi]
        per_core.append({
            "xT": np.ascontiguousarray(xs.T).astype(np.float16),
            "W1": W1p, "W2": W2p,
            "B1": b1.reshape(1, 128).astype(np.float32),
            "B2": b2.reshape(1, 128).astype(np.float32),
            "CC": np.array([[c1, c2]], np.float32),
            "DSTS": dst_s, "EA": ea_s,
            "FIDX": pack16(fidx_v), "AQIX": pack16(aq_v),
        })
    return per_core


def build_nc(cfg, skips=()):
    skips = set(skips)
    nc = bacc.Bacc("TRN2", target_bir_lowering=False, num_swdge_queues=4)
    NPC, NBLK, NCH = cfg.NPC, cfg.NBLK, cfg.NCH

    xT = nc.declare_dram_parameter("xT", [128, NPC], F16, isOutput=False)
    W = {1: nc.declare_dram_parameter("W1", [128, 393], F16, isOutput=False),
         2: nc.declare_dram_parameter("W2", [128, 393], F16, isOutput=False)}
    B = {1: nc.declare_dram_parameter("B1", [1, 128], F32, isOutput=False),
         2: nc.declare_dram_parameter("B2", [1, 128], F32, isOutput=False)}
    CC = nc.declare_dram_parameter("CC", [1, 2], F32, isOutput=False)
    DSTS = nc.declare_dram_parameter("DSTS", [128, NCH], F16, isOutput=False)
    EA = nc.declare_dram_parameter("EA", [128, NCH], F16, isOutput=False)
    FIDX = nc.declare_dram_parameter("FIDX", [16, NCH * 8], I16, isOutput=False)
    AQIX = nc.declare_dram_parameter("AQIX", [16, NCH * 8], I16, isOutput=False)
    OUT2 = nc.declare_dram_parameter("out2", [NPC, 128], F16, isOutput=True)

    tabs = {L: nc.dram_tensor(f"tabs{L}", [cfg.RPC, 256], F16) for L in (1, 2)}
    tabg = {L: nc.dram_tensor(f"tabg{L}", [cfg.RTOT, 256], F16, addr_space="Shared")
            for L in (1, 2)}
    aqt = {L: nc.dram_tensor(f"aqt{L}", [cfg.RPC, 64], F32) for L in (1, 2)}

    AL = mybir.AluOpType
    AF = mybir.ActivationFunctionType

    with TileContext(nc) as tc:
        with (
            tc.tile_pool(name="const", bufs=1) as cp,
            tc.tile_pool(name="stag", bufs=4) as sp,
            tc.tile_pool(name="aqs", bufs=6) as qp,
            tc.tile_pool(name="oa", bufs=8) as op,
            tc.tile_pool(name="work", bufs=3) as wp,
            tc.tile_pool(name="pacc", bufs=4, space="PSUM") as pa,
            tc.tile_pool(name="ptab", bufs=2, space="PSUM") as pt,
            tc.tile_pool(name="pmisc", bufs=2, space="PSUM") as px,
        ):
            # ---- constants / staged inputs ----
            xT_t = cp.tile([128, NPC], F16)
            nc.sync.dma_start(out=xT_t[:], in_=xT[:])
            W_t = {L: cp.tile([128, 393], F16, tag=f"W{L}", name=f"W{L}_t") for L in (1, 2)}
            B_t = {L: cp.tile([1, 128], F32, tag=f"B{L}", name=f"B{L}_t") for L in (1, 2)}
            for L in (1, 2):
                nc.sync.dma_start(out=W_t[L][:], in_=W[L][:])
                nc.sync.dma_start(out=B_t[L][:], in_=B[L][:])
            cc_t = cp.tile([1, 2], F32)
            nc.sync.dma_start(out=cc_t[:], in_=CC[:])
            dsth = cp.tile([128, NCH], F16)
            nc.sync.dma_start(out=dsth[:], in_=DSTS[:])
            eah = cp.tile([128, NCH], F16)
            nc.sync.dma_start(out=eah[:], in_=EA[:])
            fidx_t = cp.tile([128, NCH * 8], I16)
            aqix_t = cp.tile([128, NCH * 8], I16)
            for g in range(8):
                nc.sync.dma_start(out=fidx_t[16 * g:16 * g + 16, :], in_=FIDX[:])
                nc.sync.dma_start(out=aqix_t[16 * g:16 * g + 16, :], in_=AQIX[:])

            dst_t = cp.tile([128, NCH], F32)
            nc.vector.tensor_copy(dst_t[:], dsth[:])
            eaf = cp.tile([128, NCH], F32)
            nc.vector.tensor_copy(eaf[:], eah[:])
            et_t = cp.tile([128, NCH], F32)

            ii = cp.tile([128, 128], I32)
            nc.gpsimd.iota(ii[:], pattern=[[1, 128]], base=0, channel_multiplier=0)
            iof = cp.tile([128, 128], F32)
            nc.vector.tensor_copy(iof[:], ii[:])
            ident = cp.tile([128, 128], F16)
            make_identity(nc, ident[:])
            ones1 = cp.tile([1, 128], F32)
            nc.vector.memset(ones1[:], 1.0)

            out_sb = cp.tile([128, NBLK * 129], F32)
            h_all = cp.tile([128, NBLK * 128], F16)
            aq_all = cp.tile([128, 3 * NBLK], F32)
            bias_bc = cp.tile([128, 128], F32)

            # broadcast CC across partitions: [128, 2]
            pcc = px.tile([128, 2], F32, tag="pmisc")
            nc.tensor.matmul(pcc[:], lhsT=ones1[:], rhs=cc_t[:], start=True, stop=True)
            ccb = cp.tile([128, 2], F32)
            nc.vector.tensor_copy(ccb[:], pcc[:])

            qrr = [0]

            def qn():
                qrr[0] = (qrr[0] + 1) % 4
                return qrr[0]

            for L in (1, 2):
                # ---- per-layer edge-attr logit term and bias broadcast ----
                nc.vector.tensor_scalar_mul(et_t[:], eaf[:], ccb[:, L - 1:L])
                pb = px.tile([128, 128], F32, tag="pmisc")
                nc.tensor.matmul(pb[:], lhsT=ones1[:], rhs=B_t[L][:], start=True, stop=True)
                nc.vector.tensor_copy(bias_bc[:], pb[:])

                # ---- node transform table build ----
                for t in range(NBLK):
                    if L == 1:
                        lhs = xT_t[:, t * 128:(t + 1) * 128]
                    else:
                        pT = px.tile([128, 128], F16, tag="pmisc")
                        nc.tensor.transpose(pT[:], h_all[:, t * 128:(t + 1) * 128], ident[:])
                        hT = wp.tile([128, 128], F16, tag="hT")
                        nc.vector.tensor_copy(hT[:], pT[:])
                        lhs = hT[:]
                    ptab = pt.tile([128, 393], F32)
                    nc.tensor.matmul(ptab[:], lhsT=lhs, rhs=W_t[L][:], start=True, stop=True)
                    stab = wp.tile([128, 390], F16, tag="stab")
                    nc.vector.tensor_copy(stab[:], ptab[:, 0:390])
                    for r in range(3):
                        nc.vector.memset(stab[:, r * 130 + 128:r * 130 + 129], 1.0)
                        nc.vector.tensor_copy(aq_all[:, r * NBLK + t:r * NBLK + t + 1],
                                              ptab[:, 390 + r:391 + r])
                    for r in range(3):
                        nc.sync.dma_start(
                            out=tabs[L][r * NPC + t * 128:r * NPC + (t + 1) * 128, 0:130],
                            in_=stab[:, r * 130:r * 130 + 130])
                for r in range(3):
                    dstv = aqt[L][r * NPC:(r + 1) * NPC, 0:1] \
                        .rearrange("(t p) o -> p (t o)", p=128)
                    nc.sync.dma_start(out=dstv, in_=aq_all[:, r * NBLK:(r + 1) * NBLK])

                # ---- AllGather the table ----
                nc.gpsimd.collective_compute(
                    "AllGather", AL.bypass, replica_groups=[list(range(cfg.NC))],
                    ins=[tabs[L][:]], outs=[tabg[L][:]])

                # ---- main edge loop ----
                nc.vector.memset(out_sb[:], 0.0)
                call_tiles = {}
                expa_tiles = {}
                for (p, s0, ns) in cfg.calls:
                    vrows = min(cfg.RANGE, cfg.RTOT - p * cfg.RANGE)
                    fst = sp.tile([128, cfg.GCALL, 130], F16, tag="fst")
                    if 'gather' in skips:
                        nc.vector.memset(fst[:, 0, 0:2], 0.0)
                    else: nc.gpsimd.dma_gather(
                        fst[:, :ns, :],
                        tabg[L][p * cfg.RANGE:p * cfg.RANGE + vrows, 0:130],
                        fidx_t[:, s0 * 8:(s0 + ns) * 8],
                        ns * 128, ns * 128, 130, elem_step=256,
                        single_packet=False, queue_num=qn())
                    aqs = qp.tile([128, cfg.GCALL, 1], F32, tag="aqs")
                    if 'aq' in skips:
                        nc.vector.memset(aqs[:, 0, 0:1], 0.0)
                    else: nc.gpsimd.dma_gather(
                        aqs[:, :ns, :], aqt[L][:, 0:1],
                        aqix_t[:, s0 * 8:(s0 + ns) * 8],
                        ns * 128, ns * 128, 1, elem_step=64,
                        single_packet=False, queue_num=qn())
                    ext = qp.tile([128, cfg.GCALL], F32, tag="ext")
                    sl = ext[:, :ns]
                    if 'alpha' in skips:
                        nc.vector.memset(ext[:, 0:2], 0.0)
                    if 'alpha' not in skips:
                        nc.vector.tensor_tensor(sl, aqs[:, :ns, 0], fst[:, :ns, 129], op=AL.add)
                        nc.vector.tensor_tensor(sl, sl, et_t[:, s0:s0 + ns], op=AL.add)
                        lrt = wp.tile([128, cfg.GCALL], F32, tag="lrt")
                        nc.vector.tensor_scalar_mul(lrt[:, :ns], sl, NEG_SLOPE)
                        nc.vector.tensor_tensor(sl, sl, lrt[:, :ns], op=AL.max)
                        nc.scalar.activation(sl, sl, AF.Exp)
                    for k in range(ns):
                        call_tiles[s0 + k] = (fst, k)
                        expa_tiles[s0 + k] = (ext, k)

                for grp in [(p,) for p in range(cfg.NPH)]:
                    for b in range(NBLK):
                        slots = [int(cfg.base[p] + cfg.pboff[p, b] + c)
                                 for p in grp for c in range(int(cfg.CPB[b, p]))]
                        if not slots:
                            continue
                        pacc = pa.tile([128, 129], F32)
                        if 'mm' in skips:
                            nc.vector.memset(pacc[:, 0:2], 0.0)
                        for ci, s in enumerate(slots):
                            fst, ls = call_tiles[s]
                            oa = op.tile([128, 128], F16, tag="oa")
                            ext, ek = expa_tiles[s]
                            if 'oa' in skips:
                                nc.vector.memset(oa[:, 0:2], 0.0)
                            if 'oa' not in skips:
                                nc.vector.tensor_scalar(
                                    oa[:], iof[:], dst_t[:, s:s + 1], ext[:, ek:ek + 1],
                                    op0=AL.is_equal, op1=AL.mult)
                            if 'mm' not in skips:
                                nc.tensor.matmul(pacc[:], lhsT=oa[:], rhs=fst[:, ls, 0:129],
                                                 start=(ci == 0), stop=(ci == len(slots) - 1))
                        if 'evac' not in skips:
                            nc.vector.tensor_tensor(out_sb[:, b * 129:(b + 1) * 129],
                                                    out_sb[:, b * 129:(b + 1) * 129],
                                                    pacc[:], op=AL.add)

                # ---- finalize ----
                for b in range(NBLK):
                    rc = wp.tile([128, 1], F32, tag="rc")
                    nc.vector.tensor_scalar_add(rc[:], out_sb[:, b * 129 + 128:b * 129 + 129],
                                                1e-16)
                    nc.vector.reciprocal(rc[:], rc[:])
                    if L == 1:
                        tgt = h_all[:, b * 128:(b + 1) * 128]
                        ft = wp.tile([128, 128], F32, tag="ft")
                    else:
                        ot = wp.tile([128, 128], F16, tag="ot")
                        tgt = ot[:]
                        ft = wp.tile([128, 128], F32, tag="ft")
                    nc.vector.tensor_scalar_mul(ft[:], out_sb[:, b * 129:b * 129 + 128], rc[:])
                    if L == 1:
                        nc.vector.tensor_tensor(ft[:], ft[:], bias_bc[:], op=AL.add)
                        nc.vector.tensor_scalar_max(tgt, ft[:], 0.0)
                    else:
                        nc.vector.tensor_tensor(tgt, ft[:], bias_bc[:], op=AL.add)
                        nc.sync.dma_start(out=OUT2[b * 128:(b + 1) * 128, :], in_=tgt)
    nc.compile()
    return nc


_CACHE = {}


def run(x, edge_index, edge_type, edge_attr, w1, q1, k1, le1, e1, b1,
        w2, q2, k2, le2, e2, b2, N=None, E=None):
    x = np.asarray(x, np.float32)
    N = x.shape[0] if N is None else N
    E = edge_index.shape[1] if E is None else E
    cfg = make_cfg(N, E)
    per_core = host_prep(cfg, x, np.asarray(edge_index), np.asarray(edge_type),
                         np.asarray(edge_attr, np.float32),
                         np.asarray(w1, np.float32), np.asarray(q1, np.float32),
                         np.asarray(k1, np.float32), np.asarray(le1, np.float32),
                         np.asarray(e1, np.float32), np.asarray(b1, np.float32),
                         np.asarray(w2, np.float32), np.asarray(q2, np.float32),
                         np.asarray(k2, np.float32), np.asarray(le2, np.float32),
                         np.asarray(e2, np.float32), np.asarray(b2, np.float32))
    key = (N, E, cfg.NCH, cfg.CPB.sum())
    if key not in _CACHE:
        _CACHE[key] = build_nc(cfg)
    nc = _CACHE[key]
    res = run_bass_kernel_spmd(nc, per_core, core_ids=list(range(cfg.NC)))
    out = np.concatenate([res.results[c]["out2"] for c in range(cfg.NC)], axis=0)
    return out[:N].astype(np.float32)


def kernel(**inputs):
    return run(
        inputs["x"], inputs["edge_index"], inputs["edge_type"], inputs["edge_attr"],
        inputs["w1"], inputs["q1"], inputs["k1"], inputs["le1"], inputs["e1"], inputs["b1"],
        inputs["w2"], inputs["q2"], inputs["k2"], inputs["le2"], inputs["e2"], inputs["b2"],
    ).astype(np.float32)


# revision 22
# speedup vs baseline: 1.6561x; 1.0231x over previous
"""Two-layer RGAT (R=3, heads=1) on 8 trn2 NeuronCores.

Strategy (dst-sharded, one-hot-matmul aggregation), I/O-lean variant:
  - Nodes padded to 50176 = 8 cores x 49 blocks x 128; core c owns dst nodes
    [c*6272, (c+1)*6272) and computes the full output rows for them.
  - Per layer, each core computes its slice of the per-relation node transform
    xw[r] = x @ W_r (plus attention scalars ak = xw@k, aq = xw@q) into a DRAM
    table (row = (src_core, rt, src_local), 256-f16 stride, 130 payload:
    [128 feats | 1.0 | ak]); AllGather replicates the table.
  - Edges (sorted by dst block, then by table-row range so int16 gather
    indices fit) are processed in 128-edge chunks: dma_gather fetches the
    chunk's source rows; alpha = exp(LeakyRelu(aq[rt,dst] + ak[rt,src] +
    c_l*ea)) is built from a second (local) aq-table gather; a fused DVE
    tensor_scalar builds the alpha-scaled one-hot O[e, dst_local]; one
    matmul per chunk accumulates psum[node,129] = [sum alpha*xj | sum alpha].
  - Block results accumulate in SBUF across range-phases; finalize divides by
    the denominator, adds bias (+ReLU for layer 1). Layer-2 output rows DMA
    straight to the per-core output; the host concatenates and trims.

  Host<->device traffic is the wall-clock bottleneck (axon tunnel), so all
  bulk tensors are f16 and index tiles ship unreplicated ([16, NCH*8]; the
  gpsimd gather wants them replicated across the 8 16-partition groups, which
  the kernel does on-device with 8 DMAs). Per-edge scalars ship once (ea as
  f16; c1*ea / c2*ea are formed on-device from a [1,2] constant input).
"""
import sys
sys.path.insert(0, '/opt/trn_rl_repo')
import inspect
import textwrap
import numpy as np

import concourse.bass as bass
import concourse.bacc as bacc
import concourse.mybir as mybir
from concourse.bass_utils import run_bass_kernel_spmd
from concourse.tile import TileContext
from concourse.masks import make_identity

F32 = mybir.dt.float32
F16 = mybir.dt.float16
F8 = mybir.dt.float8e4
I16 = mybir.dt.int16
I8 = mybir.dt.int8
I32 = mybir.dt.int32
NEG_SLOPE = 0.2
import ml_dtypes

# ---- relax dma_gather's elem_size%256 restriction (descriptor length is ----
# ---- arbitrary; only the row *stride* must be a multiple of 256B)       ----
_src = inspect.getsource(bass.BassGpSimd.dma_gather)
_src = _src.replace(
    "elem_size_bytes > 0 and elem_size_bytes % 256 == 0",
    "elem_size_bytes > 0",
)
_ns = {}
exec(compile(textwrap.dedent(_src), "<dma_gather_patched>", "exec"), dict(vars(bass)), _ns)
bass.BassGpSimd.dma_gather = _ns["dma_gather"]


class Cfg:
    pass


def make_cfg(N, E, NC=8, GCALL=32, RANGE=32768):
    cfg = Cfg()
    cfg.NC = NC
    cfg.N, cfg.E = N, E
    cfg.NPAD = -(-N // (128 * NC)) * 128 * NC
    cfg.NPC = cfg.NPAD // NC
    cfg.NBLK = cfg.NPC // 128
    cfg.RPC = 3 * cfg.NPC
    cfg.RTOT = cfg.RPC * NC
    cfg.RANGE = RANGE
    cfg.NPH = -(-cfg.RTOT // RANGE)
    cfg.GCALL = GCALL
    return cfg


def host_prep(cfg, x, edge_index, edge_type, edge_attr, w1, q1, k1, le1, e1, b1,
              w2, q2, k2, le2, e2, b2):
    """Returns (per_core_inputs list, cfg with CP/calls/NCH set)."""
    NC, NPC, NBLK, RANGE = cfg.NC, cfg.NPC, cfg.NBLK, cfg.RANGE
    src, dst = edge_index[0].astype(np.int64), edge_index[1].astype(np.int64)
    rt = edge_type.astype(np.int64)
    ea = edge_attr[:, 0].astype(np.float32)
    c1 = float(le1.reshape(-1) @ e1.reshape(-1))
    c2 = float(le2.reshape(-1) @ e2.reshape(-1))

    core = dst // NPC
    blk = (dst % NPC) // 128
    dl = dst % 128
    grow = (src // NPC) * cfg.RPC + rt * NPC + (src % NPC)
    ph = grow // RANGE
    lidx = grow - ph * RANGE
    aqi = rt * NPC + (dst % NPC)

    # per (core, blk, phase) counts -> CPB[p][b] = max-over-cores chunks
    counts = np.zeros((NC, NBLK, cfg.NPH), np.int64)
    np.add.at(counts, (core, blk, ph), 1)
    CPB = -(-counts.max(axis=0) // 128)          # [NBLK, NPH]
    cfg.CPB = CPB
    # slot layout: phase-major; within phase, blocks at cumsum offsets
    cfg.pboff = np.zeros((cfg.NPH, NBLK), np.int64)
    base = [0]
    for p in range(cfg.NPH):
        cfg.pboff[p] = np.concatenate([[0], np.cumsum(CPB[:-1, p])])
        base.append(base[-1] + int(CPB[:, p].sum()))
    cfg.base = np.asarray(base, np.int64)
    cfg.NCH = int(cfg.base[-1])

    # gather call list: per phase, contiguous slot runs of <= GCALL slots
    calls = []
    for p in range(cfg.NPH):
        nslots = int(CPB[:, p].sum())
        s = 0
        while s < nslots:
            ns = min(cfg.GCALL, nslots - s)
            calls.append((p, int(cfg.base[p] + s), int(ns)))
            s += ns
    cfg.calls = calls

    def pack16(vals):
        """vals [NCH*128] -> unreplicated idx tile [16, NCH*8].

        dma_gather wants [128, NCH*8] with the 16-row pattern replicated
        across the 8 gpsimd core groups; the kernel replicates on-device.
        Layout: value for (slot s, lane l) sits at [l%16, 8*s + l//16].
        """
        v = vals.reshape(cfg.NCH, 128)          # [s, l]
        out = v.reshape(cfg.NCH, 8, 16)         # [s, l//16, l%16]
        return np.ascontiguousarray(
            out.transpose(2, 0, 1).reshape(16, cfg.NCH * 8)).astype(np.int16)

    # weight packs: per-relation weights + aq columns (ak is recomputed
    # on-device per edge as xw . k, since k is layer-global)
    def wpack(w, qv):
        W = np.zeros((128, 387), np.float32)
        for r in range(3):
            W[:, r * 128:r * 128 + 128] = w[r]
            W[:, 384 + r] = (w[r] @ qv).ravel()
        return W.astype(np.float16)

    W1p, W2p = wpack(w1, q1), wpack(w2, q2)

    per_core = []
    for c in range(NC):
        m = core == c
        eb, ep = blk[m], ph[m]
        edl, elx, eaq = dl[m], lidx[m], aqi[m]
        eea = ea[m]
        order = np.lexsort((ep, eb))
        eb, ep, edl, elx, eaq, eea = (a[order] for a in (eb, ep, edl, elx, eaq, eea))
        # rank within (blk, phase) group
        gid = eb * cfg.NPH + ep
        sortg = np.argsort(gid, kind='stable')
        assert (sortg == np.arange(len(gid))).all()  # already sorted
        boundaries = np.concatenate([[0], np.cumsum(np.bincount(gid.astype(np.int64),
                                                                minlength=NBLK * cfg.NPH))])
        rank = np.arange(len(gid)) - boundaries[gid]
        slot = cfg.base[ep] + cfg.pboff[ep, eb] + rank // 128
        prow = rank % 128

        dst_s = np.full((128, cfg.NCH), -1, np.int8)
        ea_s = np.zeros((128, cfg.NCH), ml_dtypes.float8_e4m3)
        fidx_v = np.zeros(cfg.NCH * 128, np.int64)
        aq_v = np.zeros(cfg.NCH * 128, np.int64)
        dst_s[prow, slot] = edl
        ea_s[prow, slot] = eea
        fidx_v[slot * 128 + prow] = elx
        aq_v[slot * 128 + prow] = eaq

        xs = np.zeros((cfg.NPC, x.shape[1]), np.float32)
        lo, hi = c * NPC, min((c + 1) * NPC, cfg.N)
        if hi > lo:
            xs[:hi - lo] = x[lo:hi]
        per_core.append({
            "xT": np.ascontiguousarray(xs.T).astype(np.float16),
            "W1": W1p, "W2": W2p,
            "K1": k1.reshape(1, 128).astype(np.float32),
            "K2": k2.reshape(1, 128).astype(np.float32),
            "B1": b1.reshape(1, 128).astype(np.float32),
            "B2": b2.reshape(1, 128).astype(np.float32),
            "CC": np.array([[c1, c2]], np.float32),
            "DSTS": dst_s, "EA": ea_s,
            "FIDX": pack16(fidx_v), "AQIX": pack16(aq_v),
        })
    return per_core


def build_nc(cfg, skips=()):
    skips = set(skips)
    nc = bacc.Bacc("TRN2", target_bir_lowering=False, num_swdge_queues=4)
    NPC, NBLK, NCH = cfg.NPC, cfg.NBLK, cfg.NCH

    xT = nc.declare_dram_parameter("xT", [128, NPC], F16, isOutput=False)
    W = {1: nc.declare_dram_parameter("W1", [128, 387], F16, isOutput=False),
         2: nc.declare_dram_parameter("W2", [128, 387], F16, isOutput=False)}
    K = {1: nc.declare_dram_parameter("K1", [1, 128], F32, isOutput=False),
         2: nc.declare_dram_parameter("K2", [1, 128], F32, isOutput=False)}
    B = {1: nc.declare_dram_parameter("B1", [1, 128], F32, isOutput=False),
         2: nc.declare_dram_parameter("B2", [1, 128], F32, isOutput=False)}
    CC = nc.declare_dram_parameter("CC", [1, 2], F32, isOutput=False)
    DSTS = nc.declare_dram_parameter("DSTS", [128, NCH], I8, isOutput=False)
    EA = nc.declare_dram_parameter("EA", [128, NCH], F8, isOutput=False)
    FIDX = nc.declare_dram_parameter("FIDX", [16, NCH * 8], I16, isOutput=False)
    AQIX = nc.declare_dram_parameter("AQIX", [16, NCH * 8], I16, isOutput=False)
    OUT2 = nc.declare_dram_parameter("out2", [NPC, 128], F16, isOutput=True)

    tabs = {L: nc.dram_tensor(f"tabs{L}", [cfg.RPC, 128], F16) for L in (1, 2)}
    tabg = {L: nc.dram_tensor(f"tabg{L}", [cfg.RTOT, 128], F16, addr_space="Shared")
            for L in (1, 2)}
    aqt = {L: nc.dram_tensor(f"aqt{L}", [cfg.RPC, 64], F32) for L in (1, 2)}

    AL = mybir.AluOpType
    AF = mybir.ActivationFunctionType

    with TileContext(nc) as tc:
        with (
            tc.tile_pool(name="const", bufs=1) as cp,
            tc.tile_pool(name="stag", bufs=4) as sp,
            tc.tile_pool(name="aqs", bufs=6) as qp,
            tc.tile_pool(name="oa", bufs=8) as op,
            tc.tile_pool(name="work", bufs=3) as wp,
            tc.tile_pool(name="pacc", bufs=4, space="PSUM") as pa,
            tc.tile_pool(name="pden", bufs=2, space="PSUM") as pd,
            tc.tile_pool(name="ptab", bufs=1, space="PSUM") as pt,
            tc.tile_pool(name="pmisc", bufs=1, space="PSUM") as px,
        ):
            # ---- constants / staged inputs ----
            xT_t = cp.tile([128, NPC], F16)
            nc.sync.dma_start(out=xT_t[:], in_=xT[:])
            W_t = {L: cp.tile([128, 387], F16, tag=f"W{L}", name=f"W{L}_t") for L in (1, 2)}
            K_t = {L: cp.tile([1, 128], F32, tag=f"K{L}", name=f"K{L}_t") for L in (1, 2)}
            B_t = {L: cp.tile([1, 128], F32, tag=f"B{L}", name=f"B{L}_t") for L in (1, 2)}
            for L in (1, 2):
                nc.sync.dma_start(out=W_t[L][:], in_=W[L][:])
                nc.sync.dma_start(out=K_t[L][:], in_=K[L][:])
                nc.sync.dma_start(out=B_t[L][:], in_=B[L][:])
            cc_t = cp.tile([1, 2], F32)
            nc.sync.dma_start(out=cc_t[:], in_=CC[:])
            dsth = cp.tile([128, NCH], I8)
            nc.sync.dma_start(out=dsth[:], in_=DSTS[:])
            eah = cp.tile([128, NCH], F8)
            nc.sync.dma_start(out=eah[:], in_=EA[:])
            fidx_t = cp.tile([128, NCH * 8], I16)
            aqix_t = cp.tile([128, NCH * 8], I16)
            for g in range(8):
                nc.sync.dma_start(out=fidx_t[16 * g:16 * g + 16, :], in_=FIDX[:])
                nc.sync.dma_start(out=aqix_t[16 * g:16 * g + 16, :], in_=AQIX[:])

            dst_t = cp.tile([128, NCH], F32)
            nc.vector.tensor_copy(dst_t[:], dsth[:])
            eaf = cp.tile([128, NCH], F32)
            nc.vector.tensor_copy(eaf[:], eah[:])
            et_t = cp.tile([128, NCH], F32)

            ii = cp.tile([128, 128], I32)
            nc.gpsimd.iota(ii[:], pattern=[[1, 128]], base=0, channel_multiplier=0)
            iof = cp.tile([128, 128], F32)
            nc.vector.tensor_copy(iof[:], ii[:])
            ident = cp.tile([128, 128], F16)
            make_identity(nc, ident[:])
            ones1 = cp.tile([1, 128], F32)
            nc.vector.memset(ones1[:], 1.0)
            onec = cp.tile([128, 1], F16)
            nc.vector.memset(onec[:], 1.0)
            kt3 = cp.tile([128, cfg.GCALL, 128], F16)

            out_sb = cp.tile([128, NBLK * 129], F32)
            h_all = cp.tile([128, NBLK * 128], F16)
            aq_all = cp.tile([128, 3 * NBLK], F32)
            bias_bc = cp.tile([128, 128], F32)

            # broadcast CC across partitions: [128, 2]
            pcc = px.tile([128, 2], F32, tag="pmisc")
            nc.tensor.matmul(pcc[:], lhsT=ones1[:], rhs=cc_t[:], start=True, stop=True)
            ccb = cp.tile([128, 2], F32)
            nc.vector.tensor_copy(ccb[:], pcc[:])

            qrr = [0]

            def qn():
                qrr[0] = (qrr[0] + 1) % 4
                return qrr[0]

            for L in (1, 2):
                # ---- per-layer edge-attr logit term and bias/k broadcasts ----
                nc.vector.tensor_scalar_mul(et_t[:], eaf[:], ccb[:, L - 1:L])
                pb = px.tile([128, 128], F32, tag="pmisc")
                nc.tensor.matmul(pb[:], lhsT=ones1[:], rhs=B_t[L][:], start=True, stop=True)
                nc.vector.tensor_copy(bias_bc[:], pb[:])
                pk = px.tile([128, 128], F32, tag="pmisc")
                nc.tensor.matmul(pk[:], lhsT=ones1[:], rhs=K_t[L][:], start=True, stop=True)
                for j in range(cfg.GCALL):
                    nc.vector.tensor_copy(kt3[:, j, :], pk[:])

                # ---- node transform table build ----
                for t in range(NBLK) if 'tab' not in skips else []:
                    if L == 1:
                        lhs = xT_t[:, t * 128:(t + 1) * 128]
                    else:
                        pT = px.tile([128, 128], F16, tag="pmisc")
                        nc.tensor.transpose(pT[:], h_all[:, t * 128:(t + 1) * 128], ident[:])
                        hT = wp.tile([128, 128], F16, tag="hT")
                        nc.vector.tensor_copy(hT[:], pT[:])
                        lhs = hT[:]
                    ptab = pt.tile([128, 387], F32)
                    nc.tensor.matmul(ptab[:], lhsT=lhs, rhs=W_t[L][:], start=True, stop=True)
                    stab = wp.tile([128, 384], F16, tag="stab")
                    nc.vector.tensor_copy(stab[:], ptab[:, 0:384])
                    for r in range(3):
                        nc.vector.tensor_copy(aq_all[:, r * NBLK + t:r * NBLK + t + 1],
                                              ptab[:, 384 + r:385 + r])
                    for r in range(3):
                        nc.sync.dma_start(
                            out=tabs[L][r * NPC + t * 128:r * NPC + (t + 1) * 128, :],
                            in_=stab[:, r * 128:r * 128 + 128])
                for r in range(3) if 'tab' not in skips else []:
                    dstv = aqt[L][r * NPC:(r + 1) * NPC, 0:1] \
                        .rearrange("(t p) o -> p (t o)", p=128)
                    nc.sync.dma_start(out=dstv, in_=aq_all[:, r * NBLK:(r + 1) * NBLK])

                # ---- AllGather the table ----
                if 'ag' not in skips:
                    nc.gpsimd.collective_compute(
                        "AllGather", AL.bypass, replica_groups=[list(range(cfg.NC))],
                        ins=[tabs[L][:]], outs=[tabg[L][:]])

                # ---- main edge loop ----
                nc.vector.memset(out_sb[:], 0.0)
                call_tiles = {}
                expa_tiles = {}
                for (p, s0, ns) in cfg.calls:
                    vrows = min(cfg.RANGE, cfg.RTOT - p * cfg.RANGE)
                    fst = sp.tile([128, cfg.GCALL, 128], F16, tag="fst")
                    if 'gather' in skips:
                        nc.vector.memset(fst[:, 0, 0:2], 0.0)
                    else: nc.gpsimd.dma_gather(
                        fst[:, :ns, :],
                        tabg[L][p * cfg.RANGE:p * cfg.RANGE + vrows, :],
                        fidx_t[:, s0 * 8:(s0 + ns) * 8],
                        ns * 128, ns * 128, 128, elem_step=128,
                        single_packet=False, queue_num=qn())
                    aqs = qp.tile([128, cfg.GCALL, 1], F32, tag="aqs")
                    if 'aq' in skips:
                        nc.vector.memset(aqs[:, 0, 0:1], 0.0)
                    else: nc.gpsimd.dma_gather(
                        aqs[:, :ns, :], aqt[L][:, 0:1],
                        aqix_t[:, s0 * 8:(s0 + ns) * 8],
                        ns * 128, ns * 128, 1, elem_step=64,
                        single_packet=False, queue_num=qn())
                    ext = qp.tile([128, cfg.GCALL], F32, tag="ext")
                    sl = ext[:, :ns]
                    if 'alpha' in skips:
                        nc.vector.memset(ext[:, 0:2], 0.0)
                    if 'alpha' not in skips:
                        # ak_e = xw_e . k  (k is layer-global, not per-relation)
                        prod = wp.tile([128, cfg.GCALL, 128], F16, tag="prod")
                        nc.vector.tensor_tensor(prod[:, :ns, :], fst[:, :ns, :],
                                                kt3[:, :ns, :], op=AL.mult)
                        akc = wp.tile([128, cfg.GCALL], F32, tag="akc")
                        nc.vector.tensor_reduce(akc[:, :ns], prod[:, :ns, :],
                                                axis=mybir.AxisListType.X, op=AL.add)
                        nc.vector.tensor_tensor(sl, aqs[:, :ns, 0], akc[:, :ns], op=AL.add)
                        nc.vector.tensor_tensor(sl, sl, et_t[:, s0:s0 + ns], op=AL.add)
                        nc.vector.scalar_tensor_tensor(sl, sl, NEG_SLOPE, sl,
                                                       op0=AL.mult, op1=AL.max)
                        nc.scalar.activation(sl, sl, AF.Exp)
                    for k in range(ns):
                        call_tiles[s0 + k] = (fst, k)
                        expa_tiles[s0 + k] = (ext, k)

                for grp in [(p,) for p in range(cfg.NPH)]:
                    for b in range(NBLK):
                        slots = [int(cfg.base[p] + cfg.pboff[p, b] + c)
                                 for p in grp for c in range(int(cfg.CPB[b, p]))]
                        if not slots:
                            continue
                        pacc = pa.tile([128, 128], F32)
                        pden = pd.tile([128, 1], F32)
                        if 'mm' in skips:
                            nc.vector.memset(pacc[:, 0:2], 0.0)
                            nc.vector.memset(pden[:, 0:1], 0.0)
                        for ci, s in enumerate(slots):
                            fst, ls = call_tiles[s]
                            oa = op.tile([128, 128], F16, tag="oa")
                            ext, ek = expa_tiles[s]
                            if 'oa' in skips:
                                nc.vector.memset(oa[:, 0:2], 0.0)
                            if 'oa' not in skips:
                                nc.vector.tensor_scalar(
                                    oa[:], iof[:], dst_t[:, s:s + 1], ext[:, ek:ek + 1],
                                    op0=AL.is_equal, op1=AL.mult)
                            if 'mm' not in skips:
                                first, last = ci == 0, ci == len(slots) - 1
                                nc.tensor.matmul(pacc[:], lhsT=oa[:],
                                                 rhs=fst[:, ls, :], start=first, stop=last)
                                nc.tensor.matmul(pden[:], lhsT=oa[:],
                                                 rhs=onec[:], start=first, stop=last)
                        if 'evac' not in skips:
                            nc.vector.tensor_tensor(out_sb[:, b * 129:b * 129 + 128],
                                                    out_sb[:, b * 129:b * 129 + 128],
                                                    pacc[:], op=AL.add)
                            nc.vector.tensor_tensor(out_sb[:, b * 129 + 128:b * 129 + 129],
                                                    out_sb[:, b * 129 + 128:b * 129 + 129],
                                                    pden[:], op=AL.add)

                # ---- finalize ----
                for b in range(NBLK):
                    rc = wp.tile([128, 1], F32, tag="rc")
                    nc.vector.tensor_scalar_add(rc[:], out_sb[:, b * 129 + 128:b * 129 + 129],
                                                1e-16)
                    nc.vector.reciprocal(rc[:], rc[:])
                    if L == 1:
                        tgt = h_all[:, b * 128:(b + 1) * 128]
                        ft = wp.tile([128, 128], F32, tag="ft")
                    else:
                        ot = wp.tile([128, 128], F16, tag="ot")
                        tgt = ot[:]
                        ft = wp.tile([128, 128], F32, tag="ft")
                    nc.vector.tensor_scalar_mul(ft[:], out_sb[:, b * 129:b * 129 + 128], rc[:])
                    if L == 1:
                        nc.vector.tensor_tensor(ft[:], ft[:], bias_bc[:], op=AL.add)
                        nc.vector.tensor_scalar_max(tgt, ft[:], 0.0)
                    else:
                        nc.vector.tensor_tensor(tgt, ft[:], bias_bc[:], op=AL.add)
                        nc.sync.dma_start(out=OUT2[b * 128:(b + 1) * 128, :], in_=tgt)
    nc.compile()
    return nc


_CACHE = {}


def run(x, edge_index, edge_type, edge_attr, w1, q1, k1, le1, e1, b1,
        w2, q2, k2, le2, e2, b2, N=None, E=None):
    x = np.asarray(x, np.float32)
    N = x.shape[0] if N is None else N
    E = edge_index.shape[1] if E is None else E
    cfg = make_cfg(N, E)
    per_core = host_prep(cfg, x, np.asarray(edge_index), np.asarray(edge_type),
                         np.asarray(edge_attr, np.float32),
                         np.asarray(w1, np.float32), np.asarray(q1, np.float32),
                         np.asarray(k1, np.float32), np.asarray(le1, np.float32),
                         np.asarray(e1, np.float32), np.asarray(b1, np.float32),
                         np.asarray(w2, np.float32), np.asarray(q2, np.float32),
                         np.asarray(k2, np.float32), np.asarray(le2, np.float32),
                         np.asarray(e2, np.float32), np.asarray(b2, np.float32))
    key = (N, E, cfg.NCH, cfg.CPB.sum())
    if key not in _CACHE:
        _CACHE[key] = build_nc(cfg)
    nc = _CACHE[key]
    res = run_bass_kernel_spmd(nc, per_core, core_ids=list(range(cfg.NC)))
    out = np.concatenate([res.results[c]["out2"] for c in range(cfg.NC)], axis=0)
    return out[:N].astype(np.float32)


def kernel(**inputs):
    return run(
        inputs["x"], inputs["edge_index"], inputs["edge_type"], inputs["edge_attr"],
        inputs["w1"], inputs["q1"], inputs["k1"], inputs["le1"], inputs["e1"], inputs["b1"],
        inputs["w2"], inputs["q2"], inputs["k2"], inputs["le2"], inputs["e2"], inputs["b2"],
    ).astype(np.float32)


# revision 26
# speedup vs baseline: 1.6721x; 1.0096x over previous
"""Two-layer RGAT (R=3, heads=1) on 8 trn2 NeuronCores.

Strategy (dst-sharded, one-hot-matmul aggregation), I/O-lean variant:
  - Nodes padded to 50176 = 8 cores x 49 blocks x 128; core c owns dst nodes
    [c*6272, (c+1)*6272) and computes the full output rows for them.
  - Per layer, each core computes its slice of the per-relation node transform
    xw[r] = x @ W_r (plus attention scalars ak = xw@k, aq = xw@q) into a DRAM
    table (row = (src_core, rt, src_local), 256-f16 stride, 130 payload:
    [128 feats | 1.0 | ak]); AllGather replicates the table.
  - Edges (sorted by dst block, then by table-row range so int16 gather
    indices fit) are processed in 128-edge chunks: dma_gather fetches the
    chunk's source rows; alpha = exp(LeakyRelu(aq[rt,dst] + ak[rt,src] +
    c_l*ea)) is built from a second (local) aq-table gather; a fused DVE
    tensor_scalar builds the alpha-scaled one-hot O[e, dst_local]; one
    matmul per chunk accumulates psum[node,129] = [sum alpha*xj | sum alpha].
  - Block results accumulate in SBUF across range-phases; finalize divides by
    the denominator, adds bias (+ReLU for layer 1). Layer-2 output rows DMA
    straight to the per-core output; the host concatenates and trims.

  Host<->device traffic is the wall-clock bottleneck (axon tunnel), so all
  bulk tensors are f16 and index tiles ship unreplicated ([16, NCH*8]; the
  gpsimd gather wants them replicated across the 8 16-partition groups, which
  the kernel does on-device with 8 DMAs). Per-edge scalars ship once (ea as
  f16; c1*ea / c2*ea are formed on-device from a [1,2] constant input).
"""
import sys
sys.path.insert(0, '/opt/trn_rl_repo')
import inspect
import textwrap
import numpy as np

import concourse.bass as bass
import concourse.bacc as bacc
import concourse.mybir as mybir
from concourse.bass_utils import run_bass_kernel_spmd
from concourse.tile import TileContext
from concourse.masks import make_identity

F32 = mybir.dt.float32
F16 = mybir.dt.float16
F8 = mybir.dt.float8e4
I16 = mybir.dt.int16
I8 = mybir.dt.int8
I32 = mybir.dt.int32
NEG_SLOPE = 0.2
import ml_dtypes

# ---- relax dma_gather's elem_size%256 restriction (descriptor length is ----
# ---- arbitrary; only the row *stride* must be a multiple of 256B)       ----
_src = inspect.getsource(bass.BassGpSimd.dma_gather)
_src = _src.replace(
    "elem_size_bytes > 0 and elem_size_bytes % 256 == 0",
    "elem_size_bytes > 0",
)
_ns = {}
exec(compile(textwrap.dedent(_src), "<dma_gather_patched>", "exec"), dict(vars(bass)), _ns)
bass.BassGpSimd.dma_gather = _ns["dma_gather"]


class Cfg:
    pass


def make_cfg(N, E, NC=8, GCALL=32, RANGE=32768):
    cfg = Cfg()
    cfg.NC = NC
    cfg.N, cfg.E = N, E
    cfg.NPAD = -(-N // (128 * NC)) * 128 * NC
    cfg.NPC = cfg.NPAD // NC
    cfg.NBLK = cfg.NPC // 128
    cfg.RPC = 3 * cfg.NPC
    cfg.RTOT = cfg.RPC * NC
    cfg.RANGE = RANGE
    cfg.NPH = -(-cfg.RTOT // RANGE)
    cfg.GCALL = GCALL
    return cfg


def host_prep(cfg, x, edge_index, edge_type, edge_attr, w1, q1, k1, le1, e1, b1,
              w2, q2, k2, le2, e2, b2):
    """Returns (per_core_inputs list, cfg with CP/calls/NCH set)."""
    NC, NPC, NBLK, RANGE = cfg.NC, cfg.NPC, cfg.NBLK, cfg.RANGE
    src, dst = edge_index[0].astype(np.int64), edge_index[1].astype(np.int64)
    rt = edge_type.astype(np.int64)
    ea = edge_attr[:, 0].astype(np.float32)
    c1 = float(le1.reshape(-1) @ e1.reshape(-1))
    c2 = float(le2.reshape(-1) @ e2.reshape(-1))

    core = dst // NPC
    blk = (dst % NPC) // 128
    dl = dst % 128
    grow = (src // NPC) * cfg.RPC + rt * NPC + (src % NPC)
    ph = grow // RANGE
    lidx = grow - ph * RANGE
    aqi = rt * NPC + (dst % NPC)

    # per (core, blk, phase) counts -> CPB[p][b] = max-over-cores chunks
    counts = np.zeros((NC, NBLK, cfg.NPH), np.int64)
    np.add.at(counts, (core, blk, ph), 1)
    CPB = -(-counts.max(axis=0) // 128)          # [NBLK, NPH]
    cfg.CPB = CPB
    # slot layout: phase-major; within phase, blocks at cumsum offsets
    cfg.pboff = np.zeros((cfg.NPH, NBLK), np.int64)
    base = [0]
    for p in range(cfg.NPH):
        cfg.pboff[p] = np.concatenate([[0], np.cumsum(CPB[:-1, p])])
        base.append(base[-1] + int(CPB[:, p].sum()))
    cfg.base = np.asarray(base, np.int64)
    cfg.NCH = int(cfg.base[-1])

    # gather call list: per phase, contiguous slot runs of <= GCALL slots
    calls = []
    for p in range(cfg.NPH):
        nslots = int(CPB[:, p].sum())
        s = 0
        while s < nslots:
            ns = min(cfg.GCALL, nslots - s)
            calls.append((p, int(cfg.base[p] + s), int(ns)))
            s += ns
    cfg.calls = calls

    def pack16(vals):
        """vals [NCH*128] -> unreplicated idx tile [16, NCH*8].

        dma_gather wants [128, NCH*8] with the 16-row pattern replicated
        across the 8 gpsimd core groups; the kernel replicates on-device.
        Layout: value for (slot s, lane l) sits at [l%16, 8*s + l//16].
        """
        v = vals.reshape(cfg.NCH, 128)          # [s, l]
        out = v.reshape(cfg.NCH, 8, 16)         # [s, l//16, l%16]
        return np.ascontiguousarray(
            out.transpose(2, 0, 1).reshape(16, cfg.NCH * 8)).astype(np.int16)

    # weight packs: per-relation weights + aq columns (ak is recomputed
    # on-device per edge as xw . k, since k is layer-global)
    def wpack(w, qv):
        W = np.zeros((128, 387), np.float32)
        for r in range(3):
            W[:, r * 128:r * 128 + 128] = w[r]
            W[:, 384 + r] = (w[r] @ qv).ravel()
        return W.astype(np.float16)

    W1p, W2p = wpack(w1, q1), wpack(w2, q2)

    per_core = []
    for c in range(NC):
        m = core == c
        eb, ep = blk[m], ph[m]
        edl, elx, eaq = dl[m], lidx[m], aqi[m]
        eea = ea[m]
        order = np.lexsort((ep, eb))
        eb, ep, edl, elx, eaq, eea = (a[order] for a in (eb, ep, edl, elx, eaq, eea))
        # rank within (blk, phase) group
        gid = eb * cfg.NPH + ep
        sortg = np.argsort(gid, kind='stable')
        assert (sortg == np.arange(len(gid))).all()  # already sorted
        boundaries = np.concatenate([[0], np.cumsum(np.bincount(gid.astype(np.int64),
                                                                minlength=NBLK * cfg.NPH))])
        rank = np.arange(len(gid)) - boundaries[gid]
        slot = cfg.base[ep] + cfg.pboff[ep, eb] + rank // 128
        prow = rank % 128

        dst_s = np.full((128, cfg.NCH), -1, np.int8)
        ea_s = np.zeros((128, cfg.NCH), ml_dtypes.float8_e4m3)
        fidx_v = np.zeros(cfg.NCH * 128, np.int64)
        aq_v = np.zeros(cfg.NCH * 128, np.int64)
        dst_s[prow, slot] = edl
        ea_s[prow, slot] = eea
        fidx_v[slot * 128 + prow] = elx
        aq_v[slot * 128 + prow] = eaq

        xs = np.zeros((cfg.NPC, x.shape[1]), np.float32)
        lo, hi = c * NPC, min((c + 1) * NPC, cfg.N)
        if hi > lo:
            xs[:hi - lo] = x[lo:hi]
        Wz = np.zeros_like(W1p)
        per_core.append({
            "xT": np.ascontiguousarray(xs.T).astype(np.float16),
            "W1": W1p if c == 0 else Wz, "W2": W2p if c == 0 else Wz,
            "K1": k1.reshape(1, 128).astype(np.float32),
            "K2": k2.reshape(1, 128).astype(np.float32),
            "B1": b1.reshape(1, 128).astype(np.float32),
            "B2": b2.reshape(1, 128).astype(np.float32),
            "CC": np.array([[c1, c2]], np.float32),
            "DSTS": dst_s, "EA": ea_s,
            "FIDX": pack16(fidx_v), "AQIX": pack16(aq_v),
        })
    return per_core


def build_nc(cfg, skips=()):
    skips = set(skips)
    nc = bacc.Bacc("TRN2", target_bir_lowering=False, num_swdge_queues=4)
    NPC, NBLK, NCH = cfg.NPC, cfg.NBLK, cfg.NCH

    xT = nc.declare_dram_parameter("xT", [128, NPC], F16, isOutput=False)
    W = {1: nc.declare_dram_parameter("W1", [128, 387], F16, isOutput=False),
         2: nc.declare_dram_parameter("W2", [128, 387], F16, isOutput=False)}
    K = {1: nc.declare_dram_parameter("K1", [1, 128], F32, isOutput=False),
         2: nc.declare_dram_parameter("K2", [1, 128], F32, isOutput=False)}
    B = {1: nc.declare_dram_parameter("B1", [1, 128], F32, isOutput=False),
         2: nc.declare_dram_parameter("B2", [1, 128], F32, isOutput=False)}
    CC = nc.declare_dram_parameter("CC", [1, 2], F32, isOutput=False)
    DSTS = nc.declare_dram_parameter("DSTS", [128, NCH], I8, isOutput=False)
    EA = nc.declare_dram_parameter("EA", [128, NCH], F8, isOutput=False)
    FIDX = nc.declare_dram_parameter("FIDX", [16, NCH * 8], I16, isOutput=False)
    AQIX = nc.declare_dram_parameter("AQIX", [16, NCH * 8], I16, isOutput=False)
    OUT2 = nc.declare_dram_parameter("out2", [NPC, 128], F16, isOutput=True)

    tabs = {L: nc.dram_tensor(f"tabs{L}", [cfg.RPC, 128], F16) for L in (1, 2)}
    tabg = {L: nc.dram_tensor(f"tabg{L}", [cfg.RTOT, 128], F16, addr_space="Shared")
            for L in (1, 2)}
    aqt = {L: nc.dram_tensor(f"aqt{L}", [cfg.RPC, 64], F32) for L in (1, 2)}

    AL = mybir.AluOpType
    AF = mybir.ActivationFunctionType

    with TileContext(nc) as tc:
        with (
            tc.tile_pool(name="const", bufs=1) as cp,
            tc.tile_pool(name="stag", bufs=4) as sp,
            tc.tile_pool(name="aqs", bufs=6) as qp,
            tc.tile_pool(name="oa", bufs=3) as op,
            tc.tile_pool(name="work", bufs=3) as wp,
            tc.tile_pool(name="pacc", bufs=4, space="PSUM") as pa,
            tc.tile_pool(name="pden", bufs=2, space="PSUM") as pd,
            tc.tile_pool(name="ptab", bufs=1, space="PSUM") as pt,
            tc.tile_pool(name="pmisc", bufs=1, space="PSUM") as px,
        ):
            # ---- constants / staged inputs ----
            xT_t = cp.tile([128, NPC], F16)
            nc.sync.dma_start(out=xT_t[:], in_=xT[:])
            W_t = {L: cp.tile([128, 387], F16, tag=f"W{L}", name=f"W{L}_t") for L in (1, 2)}
            K_t = {L: cp.tile([1, 128], F32, tag=f"K{L}", name=f"K{L}_t") for L in (1, 2)}
            B_t = {L: cp.tile([1, 128], F32, tag=f"B{L}", name=f"B{L}_t") for L in (1, 2)}
            for L in (1, 2):
                nc.sync.dma_start(out=W_t[L][:], in_=W[L][:])
                nc.sync.dma_start(out=K_t[L][:], in_=K[L][:])
                nc.sync.dma_start(out=B_t[L][:], in_=B[L][:])
            cc_t = cp.tile([1, 2], F32)
            nc.sync.dma_start(out=cc_t[:], in_=CC[:])
            dsth = cp.tile([128, NCH], I8)
            nc.sync.dma_start(out=dsth[:], in_=DSTS[:])
            eah = cp.tile([128, NCH], F8)
            nc.sync.dma_start(out=eah[:], in_=EA[:])
            fidx_t = cp.tile([128, NCH * 8], I16)
            aqix_t = cp.tile([128, NCH * 8], I16)
            for g in range(8):
                nc.sync.dma_start(out=fidx_t[16 * g:16 * g + 16, :], in_=FIDX[:])
                nc.sync.dma_start(out=aqix_t[16 * g:16 * g + 16, :], in_=AQIX[:])

            dst_t = cp.tile([128, NCH], F32)
            nc.vector.tensor_copy(dst_t[:], dsth[:])
            eaf = cp.tile([128, NCH], F32)
            nc.vector.tensor_copy(eaf[:], eah[:])
            et_t = cp.tile([128, NCH], F32)

            ii = cp.tile([128, 128], I32)
            nc.gpsimd.iota(ii[:], pattern=[[1, 128]], base=0, channel_multiplier=0)
            iof = cp.tile([128, 128], F32)
            nc.vector.tensor_copy(iof[:], ii[:])
            ident = cp.tile([128, 128], F16)
            make_identity(nc, ident[:])
            ones1 = cp.tile([1, 128], F32)
            nc.vector.memset(ones1[:], 1.0)
            onec = cp.tile([128, 1], F16)
            nc.vector.memset(onec[:], 1.0)
            kt3 = cp.tile([128, cfg.GCALL, 128], F16)

            out_sb = cp.tile([128, NBLK * 129], F32)
            h_all = cp.tile([128, NBLK * 128], F16)
            aq_all = cp.tile([128, 3 * NBLK], F32)
            bias_bc = cp.tile([128, 128], F32)

            # broadcast CC across partitions: [128, 2]
            pcc = px.tile([128, 2], F32, tag="pmisc")
            nc.tensor.matmul(pcc[:], lhsT=ones1[:], rhs=cc_t[:], start=True, stop=True)
            ccb = cp.tile([128, 2], F32)
            nc.vector.tensor_copy(ccb[:], pcc[:])

            qrr = [0]

            def qn():
                qrr[0] = (qrr[0] + 1) % 4
                return qrr[0]

            for L in (1, 2):
                # ---- per-layer edge-attr logit term and bias/k broadcasts ----
                nc.vector.tensor_scalar_mul(et_t[:], eaf[:], ccb[:, L - 1:L])
                pb = px.tile([128, 128], F32, tag="pmisc")
                nc.tensor.matmul(pb[:], lhsT=ones1[:], rhs=B_t[L][:], start=True, stop=True)
                nc.vector.tensor_copy(bias_bc[:], pb[:])
                pk = px.tile([128, 128], F32, tag="pmisc")
                nc.tensor.matmul(pk[:], lhsT=ones1[:], rhs=K_t[L][:], start=True, stop=True)
                for j in range(cfg.GCALL):
                    nc.vector.tensor_copy(kt3[:, j, :], pk[:])

                # ---- node transform table build ----
                for t in range(NBLK) if 'tab' not in skips else []:
                    if L == 1:
                        lhs = xT_t[:, t * 128:(t + 1) * 128]
                    else:
                        pT = px.tile([128, 128], F16, tag="pmisc")
                        nc.tensor.transpose(pT[:], h_all[:, t * 128:(t + 1) * 128], ident[:])
                        hT = wp.tile([128, 128], F16, tag="hT")
                        nc.vector.tensor_copy(hT[:], pT[:])
                        lhs = hT[:]
                    ptab = pt.tile([128, 387], F32)
                    nc.tensor.matmul(ptab[:], lhsT=lhs, rhs=W_t[L][:], start=True, stop=True)
                    stab = wp.tile([128, 384], F16, tag="stab")
                    nc.vector.tensor_copy(stab[:], ptab[:, 0:384])
                    for r in range(3):
                        nc.vector.tensor_copy(aq_all[:, r * NBLK + t:r * NBLK + t + 1],
                                              ptab[:, 384 + r:385 + r])
                    for r in range(3):
                        nc.sync.dma_start(
                            out=tabs[L][r * NPC + t * 128:r * NPC + (t + 1) * 128, :],
                            in_=stab[:, r * 128:r * 128 + 128])
                for r in range(3) if 'tab' not in skips else []:
                    dstv = aqt[L][r * NPC:(r + 1) * NPC, 0:1] \
                        .rearrange("(t p) o -> p (t o)", p=128)
                    nc.sync.dma_start(out=dstv, in_=aq_all[:, r * NBLK:(r + 1) * NBLK])

                # ---- AllGather the table ----
                if 'ag' not in skips:
                    nc.gpsimd.collective_compute(
                        "AllGather", AL.bypass, replica_groups=[list(range(cfg.NC))],
                        ins=[tabs[L][:]], outs=[tabg[L][:]])

                # ---- main edge loop ----
                nc.vector.memset(out_sb[:], 0.0)
                call_tiles = {}
                expa_tiles = {}
                for (p, s0, ns) in cfg.calls:
                    vrows = min(cfg.RANGE, cfg.RTOT - p * cfg.RANGE)
                    fst = sp.tile([128, cfg.GCALL, 128], F16, tag="fst")
                    if 'gather' in skips:
                        nc.vector.memset(fst[:, 0, 0:2], 0.0)
                    else: nc.gpsimd.dma_gather(
                        fst[:, :ns, :],
                        tabg[L][p * cfg.RANGE:p * cfg.RANGE + vrows, :],
                        fidx_t[:, s0 * 8:(s0 + ns) * 8],
                        ns * 128, ns * 128, 128, elem_step=128,
                        single_packet=False, queue_num=qn())
                    aqs = qp.tile([128, cfg.GCALL, 1], F32, tag="aqs")
                    if 'aq' in skips:
                        nc.vector.memset(aqs[:, 0, 0:1], 0.0)
                    else: nc.gpsimd.dma_gather(
                        aqs[:, :ns, :], aqt[L][:, 0:1],
                        aqix_t[:, s0 * 8:(s0 + ns) * 8],
                        ns * 128, ns * 128, 1, elem_step=64,
                        single_packet=False, queue_num=qn())
                    ext = qp.tile([128, cfg.GCALL], F32, tag="ext")
                    sl = ext[:, :ns]
                    if 'alpha' in skips:
                        nc.vector.memset(ext[:, 0:2], 0.0)
                    if 'alpha' not in skips:
                        # ak_e = xw_e . k  (k is layer-global, not per-relation)
                        prod = wp.tile([128, cfg.GCALL, 128], F16, tag="prod")
                        nc.vector.tensor_tensor(prod[:, :ns, :], fst[:, :ns, :],
                                                kt3[:, :ns, :], op=AL.mult)
                        akc = wp.tile([128, cfg.GCALL], F32, tag="akc")
                        nc.vector.tensor_reduce(akc[:, :ns], prod[:, :ns, :],
                                                axis=mybir.AxisListType.X, op=AL.add)
                        nc.vector.tensor_tensor(sl, aqs[:, :ns, 0], akc[:, :ns], op=AL.add)
                        nc.vector.tensor_tensor(sl, sl, et_t[:, s0:s0 + ns], op=AL.add)
                        nc.vector.scalar_tensor_tensor(sl, sl, NEG_SLOPE, sl,
                                                       op0=AL.mult, op1=AL.max)
                        nc.scalar.activation(sl, sl, AF.Exp)
                    # batched alpha-scaled one-hot for all ns slots of the call
                    oa3 = op.tile([128, cfg.GCALL, 128], F16, tag="oa3")
                    if 'oa' in skips:
                        nc.vector.memset(oa3[:, 0, 0:2], 0.0)
                    else:
                        iof_b = iof[:].rearrange("p (o f) -> p o f", o=1) \
                            .broadcast_to([128, ns, 128])
                        dst_b = dst_t[:, s0:s0 + ns].rearrange("p (s o) -> p s o", o=1) \
                            .broadcast_to([128, ns, 128])
                        ext_b = ext[:, :ns].rearrange("p (s o) -> p s o", o=1) \
                            .broadcast_to([128, ns, 128])
                        nc.vector.tensor_tensor(oa3[:, :ns, :], iof_b, dst_b,
                                                op=AL.is_equal)
                        nc.vector.tensor_tensor(oa3[:, :ns, :], oa3[:, :ns, :], ext_b,
                                                op=AL.mult)
                    for k in range(ns):
                        call_tiles[s0 + k] = (fst, k)
                        expa_tiles[s0 + k] = (oa3, k)

                for grp in [(p,) for p in range(cfg.NPH)]:
                    for b in range(NBLK):
                        slots = [int(cfg.base[p] + cfg.pboff[p, b] + c)
                                 for p in grp for c in range(int(cfg.CPB[b, p]))]
                        if not slots:
                            continue
                        pacc = pa.tile([128, 128], F32)
                        pden = pd.tile([128, 1], F32)
                        if 'mm' in skips:
                            nc.vector.memset(pacc[:, 0:2], 0.0)
                            nc.vector.memset(pden[:, 0:1], 0.0)
                        for ci, s in enumerate(slots):
                            fst, ls = call_tiles[s]
                            oa3, ek = expa_tiles[s]
                            if 'mm' not in skips:
                                first, last = ci == 0, ci == len(slots) - 1
                                nc.tensor.matmul(pacc[:], lhsT=oa3[:, ek, :],
                                                 rhs=fst[:, ls, :], start=first, stop=last)
                                nc.tensor.matmul(pden[:], lhsT=oa3[:, ek, :],
                                                 rhs=onec[:], start=first, stop=last)
                        if 'evac' not in skips:
                            nc.vector.tensor_tensor(out_sb[:, b * 129:b * 129 + 128],
                                                    out_sb[:, b * 129:b * 129 + 128],
                                                    pacc[:], op=AL.add)
                            nc.vector.tensor_tensor(out_sb[:, b * 129 + 128:b * 129 + 129],
                                                    out_sb[:, b * 129 + 128:b * 129 + 129],
                                                    pden[:], op=AL.add)

                # ---- finalize ----
                for b in range(NBLK):
                    rc = wp.tile([128, 1], F32, tag="rc")
                    nc.vector.tensor_scalar_add(rc[:], out_sb[:, b * 129 + 128:b * 129 + 129],
                                                1e-16)
                    nc.vector.reciprocal(rc[:], rc[:])
                    if L == 1:
                        tgt = h_all[:, b * 128:(b + 1) * 128]
                        ft = wp.tile([128, 128], F32, tag="ft")
                    else:
                        ot = wp.tile([128, 128], F16, tag="ot")
                        tgt = ot[:]
                        ft = wp.tile([128, 128], F32, tag="ft")
                    nc.vector.tensor_scalar_mul(ft[:], out_sb[:, b * 129:b * 129 + 128], rc[:])
                    if L == 1:
                        nc.vector.tensor_tensor(ft[:], ft[:], bias_bc[:], op=AL.add)
                        nc.vector.tensor_scalar_max(tgt, ft[:], 0.0)
                    else:
                        nc.vector.tensor_tensor(tgt, ft[:], bias_bc[:], op=AL.add)
                        nc.sync.dma_start(out=OUT2[b * 128:(b + 1) * 128, :], in_=tgt)
    nc.compile()
    return nc


_CACHE = {}


def run(x, edge_index, edge_type, edge_attr, w1, q1, k1, le1, e1, b1,
        w2, q2, k2, le2, e2, b2, N=None, E=None):
    x = np.asarray(x, np.float32)
    N = x.shape[0] if N is None else N
    E = edge_index.shape[1] if E is None else E
    cfg = make_cfg(N, E)
    per_core = host_prep(cfg, x, np.asarray(edge_index), np.asarray(edge_type),
                         np.asarray(edge_attr, np.float32),
                         np.asarray(w1, np.float32), np.asarray(q1, np.float32),
                         np.asarray(k1, np.float32), np.asarray(le1, np.float32),
                         np.asarray(e1, np.float32), np.asarray(b1, np.float32),
                         np.asarray(w2, np.float32), np.asarray(q2, np.float32),
                         np.asarray(k2, np.float32), np.asarray(le2, np.float32),
                         np.asarray(e2, np.float32), np.asarray(b2, np.float32))
    key = (N, E, cfg.NCH, cfg.CPB.sum())
    if key not in _CACHE:
        _CACHE[key] = build_nc(cfg)
    nc = _CACHE[key]
    res = run_bass_kernel_spmd(nc, per_core, core_ids=list(range(cfg.NC)))
    out = np.concatenate([res.results[c]["out2"] for c in range(cfg.NC)], axis=0)
    return out[:N].astype(np.float32)


def kernel(**inputs):
    return run(
        inputs["x"], inputs["edge_index"], inputs["edge_type"], inputs["edge_attr"],
        inputs["w1"], inputs["q1"], inputs["k1"], inputs["le1"], inputs["e1"], inputs["b1"],
        inputs["w2"], inputs["q2"], inputs["k2"], inputs["le2"], inputs["e2"], inputs["b2"],
    ).astype(np.float32)


# revision 35
# speedup vs baseline: 1.7920x; 1.0717x over previous
"""Two-layer RGAT (R=3, heads=1) on 8 trn2 NeuronCores.

Strategy (dst-sharded, one-hot-matmul aggregation), I/O-lean variant:
  - Nodes padded to 50176 = 8 cores x 49 blocks x 128; core c owns dst nodes
    [c*6272, (c+1)*6272) and computes the full output rows for them.
  - Per layer, each core computes its slice of the per-relation node transform
    xw[r] = x @ W_r (plus the aq = xw@q attention scalar) into a DRAM table
    (row = (src_core, rt, src_local), exactly 128 f16 = 256B per row);
    AllGather replicates the table. ak = xw@k is NOT tabulated: k is
    layer-global, so ak is recomputed per edge from the gathered row.
  - Edges (sorted by dst block, then by table-row range so int16 gather
    indices fit) are processed in 128-edge chunks: dma_gather fetches the
    chunk's source rows; alpha = exp(LeakyRelu(aq[rt,dst] + xw_src.k +
    c_l*ea)) uses a second (local) aq-table gather; two broadcast DVE ops
    per 32-chunk call build all alpha-scaled one-hots O[e, dst_local]; two
    matmuls per chunk accumulate psum feats [node,128] and denom [node,1].
  - Block results accumulate in SBUF across range-phases; finalize divides by
    the denominator, adds bias (+ReLU for layer 1). Layer-2 output rows DMA
    straight to the per-core output (f16); the host concatenates and trims.

  Host<->device traffic is the wall-clock bottleneck (axon tunnel ~50MB/s),
  so all bulk tensors ship in the smallest workable dtype (x/out f16, ea
  f8e4m3, dst-local int8, gather indices int16) and index tiles ship
  unreplicated ([16, NCH*8]; the gpsimd gather wants them replicated across
  the 8 16-partition groups, which the kernel does on-device with 8 DMAs).
  Replicated weights ship on core 0 only (zeros elsewhere) and are
  broadcast on-device via AllGather.
"""
import sys
sys.path.insert(0, '/opt/trn_rl_repo')
import inspect
import textwrap
import numpy as np

import concourse.bass as bass
import concourse.bacc as bacc
import concourse.mybir as mybir
from concourse.bass_utils import run_bass_kernel_spmd
from concourse.tile import TileContext
from concourse.masks import make_identity

F32 = mybir.dt.float32
F16 = mybir.dt.float16
F8 = mybir.dt.float8e4
I16 = mybir.dt.int16
I8 = mybir.dt.int8
I32 = mybir.dt.int32
NEG_SLOPE = 0.2
import ml_dtypes

# ---- relax dma_gather's elem_size%256 restriction (descriptor length is ----
# ---- arbitrary; only the row *stride* must be a multiple of 256B)       ----
_src = inspect.getsource(bass.BassGpSimd.dma_gather)
_src = _src.replace(
    "elem_size_bytes > 0 and elem_size_bytes % 256 == 0",
    "elem_size_bytes > 0",
)
_ns = {}
exec(compile(textwrap.dedent(_src), "<dma_gather_patched>", "exec"), dict(vars(bass)), _ns)
bass.BassGpSimd.dma_gather = _ns["dma_gather"]


class Cfg:
    pass


def make_cfg(N, E, NC=8, GCALL=32, RANGE=32768):
    cfg = Cfg()
    cfg.NC = NC
    cfg.N, cfg.E = N, E
    cfg.NPAD = -(-N // (128 * NC)) * 128 * NC
    cfg.NPC = cfg.NPAD // NC
    cfg.NBLK = cfg.NPC // 128
    cfg.RPC = 3 * cfg.NPC
    cfg.RTOT = cfg.RPC * NC
    cfg.RANGE = RANGE
    cfg.NPH = -(-cfg.RTOT // RANGE)
    cfg.GCALL = GCALL
    return cfg


def host_prep(cfg, x, edge_index, edge_type, edge_attr, w1, q1, k1, le1, e1, b1,
              w2, q2, k2, le2, e2, b2):
    """Returns (per_core_inputs list, cfg with CP/calls/NCH set)."""
    NC, NPC, NBLK, RANGE = cfg.NC, cfg.NPC, cfg.NBLK, cfg.RANGE
    src, dst = edge_index[0].astype(np.int64), edge_index[1].astype(np.int64)
    rt = edge_type.astype(np.int64)
    ea = edge_attr[:, 0].astype(np.float32)
    c1 = float(le1.reshape(-1) @ e1.reshape(-1))
    c2 = float(le2.reshape(-1) @ e2.reshape(-1))

    core = dst // NPC
    blk = (dst % NPC) // 128
    dl = dst % 128
    grow = (src // NPC) * cfg.RPC + rt * NPC + (src % NPC)
    ph = grow // RANGE
    lidx = grow - ph * RANGE
    aqi = rt * NPC + (dst % NPC)

    # per (core, blk, phase) counts -> CPB[p][b] = max-over-cores chunks
    counts = np.zeros((NC, NBLK, cfg.NPH), np.int64)
    np.add.at(counts, (core, blk, ph), 1)
    CPB = -(-counts.max(axis=0) // 128)          # [NBLK, NPH]
    cfg.CPB = CPB
    # slot layout: phase-major; within phase, blocks at cumsum offsets
    cfg.pboff = np.zeros((cfg.NPH, NBLK), np.int64)
    base = [0]
    for p in range(cfg.NPH):
        cfg.pboff[p] = np.concatenate([[0], np.cumsum(CPB[:-1, p])])
        base.append(base[-1] + int(CPB[:, p].sum()))
    cfg.base = np.asarray(base, np.int64)
    cfg.NCH = int(cfg.base[-1])

    # gather call list: per phase, contiguous slot runs of <= GCALL slots
    calls = []
    for p in range(cfg.NPH):
        nslots = int(CPB[:, p].sum())
        s = 0
        while s < nslots:
            ns = min(cfg.GCALL, nslots - s)
            calls.append((p, int(cfg.base[p] + s), int(ns)))
            s += ns
    cfg.calls = calls

    def pack16(vals):
        """vals [NCH*128] -> unreplicated idx tile [16, NCH*8].

        dma_gather wants [128, NCH*8] with the 16-row pattern replicated
        across the 8 gpsimd core groups; the kernel replicates on-device.
        Layout: value for (slot s, lane l) sits at [l%16, 8*s + l//16].
        """
        v = vals.reshape(cfg.NCH, 128)          # [s, l]
        out = v.reshape(cfg.NCH, 8, 16)         # [s, l//16, l%16]
        return np.ascontiguousarray(
            out.transpose(2, 0, 1).reshape(16, cfg.NCH * 8)).astype(np.int16)

    # weight packs: per-relation weights + aq columns (ak is recomputed
    # on-device per edge as xw . k, since k is layer-global)
    def wpack(w, qv):
        W = np.zeros((128, 387), np.float32)
        for r in range(3):
            W[:, r * 128:r * 128 + 128] = w[r]
            W[:, 384 + r] = (w[r] @ qv).ravel()
        return W.astype(np.float16)

    W1p, W2p = wpack(w1, q1), wpack(w2, q2)

    per_core = []
    for c in range(NC):
        m = core == c
        eb, ep = blk[m], ph[m]
        edl, elx, eaq = dl[m], lidx[m], aqi[m]
        eea = ea[m]
        order = np.lexsort((ep, eb))
        eb, ep, edl, elx, eaq, eea = (a[order] for a in (eb, ep, edl, elx, eaq, eea))
        # rank within (blk, phase) group
        gid = eb * cfg.NPH + ep
        sortg = np.argsort(gid, kind='stable')
        assert (sortg == np.arange(len(gid))).all()  # already sorted
        boundaries = np.concatenate([[0], np.cumsum(np.bincount(gid.astype(np.int64),
                                                                minlength=NBLK * cfg.NPH))])
        rank = np.arange(len(gid)) - boundaries[gid]
        slot = cfg.base[ep] + cfg.pboff[ep, eb] + rank // 128
        prow = rank % 128

        dst_s = np.full((128, cfg.NCH), -1, np.int8)
        ea_s = np.zeros((128, cfg.NCH), ml_dtypes.float8_e4m3)
        fidx_v = np.zeros(cfg.NCH * 128, np.int64)
        aq_v = np.zeros(cfg.NCH * 128, np.int64)
        dst_s[prow, slot] = edl
        ea_s[prow, slot] = eea
        fidx_v[slot * 128 + prow] = elx
        aq_v[slot * 128 + prow] = eaq

        xs = np.zeros((cfg.NPC, x.shape[1]), np.float32)
        lo, hi = c * NPC, min((c + 1) * NPC, cfg.N)
        if hi > lo:
            xs[:hi - lo] = x[lo:hi]
        Wz = np.zeros_like(W1p)
        per_core.append({
            "xT": np.ascontiguousarray(xs.T).astype(np.float16),
            "W1": W1p if c == 0 else Wz, "W2": W2p if c == 0 else Wz,
            "K1": k1.reshape(1, 128).astype(np.float32),
            "K2": k2.reshape(1, 128).astype(np.float32),
            "B1": b1.reshape(1, 128).astype(np.float32),
            "B2": b2.reshape(1, 128).astype(np.float32),
            "CC": np.array([[c1, c2]], np.float32),
            "DSTS": dst_s, "EA": ea_s,
            "FIDX": pack16(fidx_v), "AQIX": pack16(aq_v),
        })
    return per_core


def build_nc(cfg, skips=()):
    skips = set(skips)
    nc = bacc.Bacc("TRN2", target_bir_lowering=False, num_swdge_queues=4)
    NPC, NBLK, NCH = cfg.NPC, cfg.NBLK, cfg.NCH

    xT = nc.declare_dram_parameter("xT", [128, NPC], F16, isOutput=False)
    W = {1: nc.declare_dram_parameter("W1", [128, 387], F16, isOutput=False),
         2: nc.declare_dram_parameter("W2", [128, 387], F16, isOutput=False)}
    K = {1: nc.declare_dram_parameter("K1", [1, 128], F32, isOutput=False),
         2: nc.declare_dram_parameter("K2", [1, 128], F32, isOutput=False)}
    B = {1: nc.declare_dram_parameter("B1", [1, 128], F32, isOutput=False),
         2: nc.declare_dram_parameter("B2", [1, 128], F32, isOutput=False)}
    CC = nc.declare_dram_parameter("CC", [1, 2], F32, isOutput=False)
    DSTS = nc.declare_dram_parameter("DSTS", [128, NCH], I8, isOutput=False)
    EA = nc.declare_dram_parameter("EA", [128, NCH], F8, isOutput=False)
    FIDX = nc.declare_dram_parameter("FIDX", [16, NCH * 8], I16, isOutput=False)
    AQIX = nc.declare_dram_parameter("AQIX", [16, NCH * 8], I16, isOutput=False)
    OUT2 = nc.declare_dram_parameter("out2", [NPC, 128], F16, isOutput=True)

    Ws = {L: nc.dram_tensor(f"Ws{L}", [128, 387], F16) for L in (1, 2)}
    Wg = {L: nc.dram_tensor(f"Wg{L}", [128 * cfg.NC, 387], F16, addr_space="Shared")
          for L in (1, 2)}
    tabs = {L: nc.dram_tensor(f"tabs{L}", [cfg.RPC, 128], F16) for L in (1, 2)}
    tabg = {L: nc.dram_tensor(f"tabg{L}", [cfg.RTOT, 128], F16, addr_space="Shared")
            for L in (1, 2)}
    aqt = {L: nc.dram_tensor(f"aqt{L}", [cfg.RPC, 64], F32) for L in (1, 2)}

    AL = mybir.AluOpType
    AF = mybir.ActivationFunctionType

    with TileContext(nc) as tc:
        with (
            tc.tile_pool(name="const", bufs=1) as cp,
            tc.tile_pool(name="stag", bufs=4) as sp,
            tc.tile_pool(name="aqs", bufs=6) as qp,
            tc.tile_pool(name="oa", bufs=3) as op,
            tc.tile_pool(name="work", bufs=3) as wp,
            tc.tile_pool(name="pacc", bufs=4, space="PSUM") as pa,
            tc.tile_pool(name="pden", bufs=2, space="PSUM") as pd,
            tc.tile_pool(name="ptab", bufs=1, space="PSUM") as pt,
            tc.tile_pool(name="pmisc", bufs=1, space="PSUM") as px,
        ):
            # ---- constants / staged inputs ----
            xT_t = cp.tile([128, NPC], F16)
            nc.sync.dma_start(out=xT_t[:], in_=xT[:])
            # W ships on core 0 only (zeros elsewhere); AllGather + read slice 0
            W_t = {L: cp.tile([128, 387], F16, tag=f"W{L}", name=f"W{L}_t") for L in (1, 2)}
            K_t = {L: cp.tile([1, 128], F32, tag=f"K{L}", name=f"K{L}_t") for L in (1, 2)}
            B_t = {L: cp.tile([1, 128], F32, tag=f"B{L}", name=f"B{L}_t") for L in (1, 2)}
            for L in (1, 2):
                nc.sync.dma_start(out=W_t[L][:], in_=W[L][:])
                nc.sync.dma_start(out=Ws[L][:], in_=W_t[L][:])
                nc.gpsimd.collective_compute(
                    "AllGather", mybir.AluOpType.bypass,
                    replica_groups=[list(range(cfg.NC))],
                    ins=[Ws[L][:]], outs=[Wg[L][:]])
                nc.sync.dma_start(out=W_t[L][:], in_=Wg[L][0:128, :])
                nc.sync.dma_start(out=K_t[L][:], in_=K[L][:])
                nc.sync.dma_start(out=B_t[L][:], in_=B[L][:])
            cc_t = cp.tile([1, 2], F32)
            nc.sync.dma_start(out=cc_t[:], in_=CC[:])
            dsth = cp.tile([128, NCH], I8)
            nc.sync.dma_start(out=dsth[:], in_=DSTS[:])
            eah = cp.tile([128, NCH], F8)
            nc.sync.dma_start(out=eah[:], in_=EA[:])
            fidx_t = cp.tile([128, NCH * 8], I16)
            aqix_t = cp.tile([128, NCH * 8], I16)
            for g in range(8):
                nc.sync.dma_start(out=fidx_t[16 * g:16 * g + 16, :], in_=FIDX[:])
                nc.sync.dma_start(out=aqix_t[16 * g:16 * g + 16, :], in_=AQIX[:])

            dst_t = cp.tile([128, NCH], F32)
            nc.vector.tensor_copy(dst_t[:], dsth[:])
            eaf = cp.tile([128, NCH], F32)
            nc.vector.tensor_copy(eaf[:], eah[:])
            et_t = cp.tile([128, NCH], F32)

            ii = cp.tile([128, 128], I32)
            nc.gpsimd.iota(ii[:], pattern=[[1, 128]], base=0, channel_multiplier=0)
            iof = cp.tile([128, 128], F32)
            nc.vector.tensor_copy(iof[:], ii[:])
            ident = cp.tile([128, 128], F16)
            make_identity(nc, ident[:])
            ones1 = cp.tile([1, 128], F32)
            nc.vector.memset(ones1[:], 1.0)
            onec = cp.tile([128, 1], F16)
            nc.vector.memset(onec[:], 1.0)
            kt3 = cp.tile([128, cfg.GCALL, 128], F16)

            out_sb = cp.tile([128, NBLK * 129], F32)
            h_all = cp.tile([128, NBLK * 128], F16)
            aq_all = cp.tile([128, 3 * NBLK], F32)
            bias_bc = cp.tile([128, 128], F32)

            # broadcast CC across partitions: [128, 2]
            pcc = px.tile([128, 2], F32, tag="pmisc")
            nc.tensor.matmul(pcc[:], lhsT=ones1[:], rhs=cc_t[:], start=True, stop=True)
            ccb = cp.tile([128, 2], F32)
            nc.vector.tensor_copy(ccb[:], pcc[:])

            qrr = [0]

            def qn():
                qrr[0] = (qrr[0] + 1) % 4
                return qrr[0]

            for L in (1, 2):
                # ---- per-layer edge-attr logit term and bias/k broadcasts ----
                nc.vector.tensor_scalar_mul(et_t[:], eaf[:], ccb[:, L - 1:L])
                pb = px.tile([128, 128], F32, tag="pmisc")
                nc.tensor.matmul(pb[:], lhsT=ones1[:], rhs=B_t[L][:], start=True, stop=True)
                nc.vector.tensor_copy(bias_bc[:], pb[:])
                pk = px.tile([128, 128], F32, tag="pmisc")
                nc.tensor.matmul(pk[:], lhsT=ones1[:], rhs=K_t[L][:], start=True, stop=True)
                for j in range(cfg.GCALL):
                    nc.vector.tensor_copy(kt3[:, j, :], pk[:])

                # ---- node transform table build ----
                for t in range(NBLK) if 'tab' not in skips else []:
                    if L == 1:
                        lhs = xT_t[:, t * 128:(t + 1) * 128]
                    else:
                        pT = px.tile([128, 128], F16, tag="pmisc")
                        nc.tensor.transpose(pT[:], h_all[:, t * 128:(t + 1) * 128], ident[:])
                        hT = wp.tile([128, 128], F16, tag="hT")
                        nc.vector.tensor_copy(hT[:], pT[:])
                        lhs = hT[:]
                    ptab = pt.tile([128, 387], F32)
                    nc.tensor.matmul(ptab[:], lhsT=lhs, rhs=W_t[L][:], start=True, stop=True)
                    stab = wp.tile([128, 384], F16, tag="stab")
                    nc.vector.tensor_copy(stab[:], ptab[:, 0:384])
                    for r in range(3):
                        nc.vector.tensor_copy(aq_all[:, r * NBLK + t:r * NBLK + t + 1],
                                              ptab[:, 384 + r:385 + r])
                    for r in range(3):
                        nc.sync.dma_start(
                            out=tabs[L][r * NPC + t * 128:r * NPC + (t + 1) * 128, :],
                            in_=stab[:, r * 128:r * 128 + 128])
                for r in range(3) if 'tab' not in skips else []:
                    dstv = aqt[L][r * NPC:(r + 1) * NPC, 0:1] \
                        .rearrange("(t p) o -> p (t o)", p=128)
                    nc.sync.dma_start(out=dstv, in_=aq_all[:, r * NBLK:(r + 1) * NBLK])

                # ---- AllGather the table ----
                if 'ag' not in skips:
                    nc.gpsimd.collective_compute(
                        "AllGather", AL.bypass, replica_groups=[list(range(cfg.NC))],
                        ins=[tabs[L][:]], outs=[tabg[L][:]])

                # ---- main edge loop ----
                nc.vector.memset(out_sb[:], 0.0)
                call_tiles = {}
                expa_tiles = {}
                for (p, s0, ns) in cfg.calls:
                    vrows = min(cfg.RANGE, cfg.RTOT - p * cfg.RANGE)
                    fst = sp.tile([128, cfg.GCALL, 128], F16, tag="fst")
                    if 'gather' in skips:
                        nc.vector.memset(fst[:, 0, 0:2], 0.0)
                    else: nc.gpsimd.dma_gather(
                        fst[:, :ns, :],
                        tabg[L][p * cfg.RANGE:p * cfg.RANGE + vrows, :],
                        fidx_t[:, s0 * 8:(s0 + ns) * 8],
                        ns * 128, ns * 128, 128, elem_step=128,
                        single_packet=False, queue_num=qn())
                    aqs = qp.tile([128, cfg.GCALL, 1], F32, tag="aqs")
                    if 'aq' in skips:
                        nc.vector.memset(aqs[:, 0, 0:1], 0.0)
                    else: nc.gpsimd.dma_gather(
                        aqs[:, :ns, :], aqt[L][:, 0:1],
                        aqix_t[:, s0 * 8:(s0 + ns) * 8],
                        ns * 128, ns * 128, 1, elem_step=64,
                        single_packet=False, queue_num=qn())
                    ext = qp.tile([128, cfg.GCALL], F32, tag="ext")
                    sl = ext[:, :ns]
                    if 'alpha' in skips:
                        nc.vector.memset(ext[:, 0:2], 0.0)
                    if 'alpha' not in skips:
                        # ak_e = xw_e . k  (k is layer-global, not per-relation)
                        prod = wp.tile([128, cfg.GCALL, 128], F16, tag="prod")
                        nc.vector.tensor_tensor(prod[:, :ns, :], fst[:, :ns, :],
                                                kt3[:, :ns, :], op=AL.mult)
                        akc = wp.tile([128, cfg.GCALL], F32, tag="akc")
                        nc.vector.tensor_reduce(akc[:, :ns], prod[:, :ns, :],
                                                axis=mybir.AxisListType.X, op=AL.add)
                        nc.vector.tensor_tensor(sl, aqs[:, :ns, 0], akc[:, :ns], op=AL.add)
                        nc.vector.tensor_tensor(sl, sl, et_t[:, s0:s0 + ns], op=AL.add)
                        nc.vector.scalar_tensor_tensor(sl, sl, NEG_SLOPE, sl,
                                                       op0=AL.mult, op1=AL.max)
                        nc.scalar.activation(sl, sl, AF.Exp)
                    # batched alpha-scaled one-hot for all ns slots of the call
                    oa3 = op.tile([128, cfg.GCALL, 128], F16, tag="oa3")
                    if 'oa' in skips:
                        nc.vector.memset(oa3[:, 0, 0:2], 0.0)
                    else:
                        iof_b = iof[:].rearrange("p (o f) -> p o f", o=1) \
                            .broadcast_to([128, ns, 128])
                        dst_b = dst_t[:, s0:s0 + ns].rearrange("p (s o) -> p s o", o=1) \
                            .broadcast_to([128, ns, 128])
                        ext_b = ext[:, :ns].rearrange("p (s o) -> p s o", o=1) \
                            .broadcast_to([128, ns, 128])
                        nc.vector.tensor_tensor(oa3[:, :ns, :], iof_b, dst_b,
                                                op=AL.is_equal)
                        nc.vector.tensor_tensor(oa3[:, :ns, :], oa3[:, :ns, :], ext_b,
                                                op=AL.mult)
                    for k in range(ns):
                        call_tiles[s0 + k] = (fst, k)
                        expa_tiles[s0 + k] = (oa3, k)

                for grp in [(p,) for p in range(cfg.NPH)]:
                    for b in range(NBLK):
                        slots = [int(cfg.base[p] + cfg.pboff[p, b] + c)
                                 for p in grp for c in range(int(cfg.CPB[b, p]))]
                        if not slots:
                            continue
                        pacc = pa.tile([128, 128], F32)
                        pden = pd.tile([128, 1], F32)
                        if 'mm' in skips:
                            nc.vector.memset(pacc[:, 0:2], 0.0)
                            nc.vector.memset(pden[:, 0:1], 0.0)
                        for ci, s in enumerate(slots):
                            fst, ls = call_tiles[s]
                            oa3, ek = expa_tiles[s]
                            if 'mm' not in skips:
                                first, last = ci == 0, ci == len(slots) - 1
                                nc.tensor.matmul(pacc[:], lhsT=oa3[:, ek, :],
                                                 rhs=fst[:, ls, :], start=first, stop=last)
                                nc.tensor.matmul(pden[:], lhsT=oa3[:, ek, :],
                                                 rhs=onec[:], start=first, stop=last)
                        if 'evac' not in skips:
                            nc.vector.tensor_tensor(out_sb[:, b * 129:b * 129 + 128],
                                                    out_sb[:, b * 129:b * 129 + 128],
                                                    pacc[:], op=AL.add)
                            nc.vector.tensor_tensor(out_sb[:, b * 129 + 128:b * 129 + 129],
                                                    out_sb[:, b * 129 + 128:b * 129 + 129],
                                                    pden[:], op=AL.add)

                # ---- finalize ----
                for b in range(NBLK):
                    rc = wp.tile([128, 1], F32, tag="rc")
                    nc.vector.tensor_scalar_add(rc[:], out_sb[:, b * 129 + 128:b * 129 + 129],
                                                1e-16)
                    nc.vector.reciprocal(rc[:], rc[:])
                    if L == 1:
                        tgt = h_all[:, b * 128:(b + 1) * 128]
                        ft = wp.tile([128, 128], F32, tag="ft")
                    else:
                        ot = wp.tile([128, 128], F16, tag="ot")
                        tgt = ot[:]
                        ft = wp.tile([128, 128], F32, tag="ft")
                    nc.vector.tensor_scalar_mul(ft[:], out_sb[:, b * 129:b * 129 + 128], rc[:])
                    if L == 1:
                        nc.vector.tensor_tensor(ft[:], ft[:], bias_bc[:], op=AL.add)
                        nc.vector.tensor_scalar_max(tgt, ft[:], 0.0)
                    else:
                        nc.vector.tensor_tensor(tgt, ft[:], bias_bc[:], op=AL.add)
                        nc.sync.dma_start(out=OUT2[b * 128:(b + 1) * 128, :], in_=tgt)
    nc.compile()
    return nc


_CACHE = {}


def run(x, edge_index, edge_type, edge_attr, w1, q1, k1, le1, e1, b1,
        w2, q2, k2, le2, e2, b2, N=None, E=None):
    x = np.asarray(x, np.float32)
    N = x.shape[0] if N is None else N
    E = edge_index.shape[1] if E is None else E
    cfg = make_cfg(N, E)
    per_core = host_prep(cfg, x, np.asarray(edge_index), np.asarray(edge_type),
                         np.asarray(edge_attr, np.float32),
                         np.asarray(w1, np.float32), np.asarray(q1, np.float32),
                         np.asarray(k1, np.float32), np.asarray(le1, np.float32),
                         np.asarray(e1, np.float32), np.asarray(b1, np.float32),
                         np.asarray(w2, np.float32), np.asarray(q2, np.float32),
                         np.asarray(k2, np.float32), np.asarray(le2, np.float32),
                         np.asarray(e2, np.float32), np.asarray(b2, np.float32))
    key = (N, E, cfg.NCH, cfg.CPB.sum())
    if key not in _CACHE:
        _CACHE[key] = build_nc(cfg)
    nc = _CACHE[key]
    res = run_bass_kernel_spmd(nc, per_core, core_ids=list(range(cfg.NC)))
    out = np.concatenate([res.results[c]["out2"] for c in range(cfg.NC)], axis=0)
    return out[:N].astype(np.float32)


def kernel(**inputs):
    return run(
        inputs["x"], inputs["edge_index"], inputs["edge_type"], inputs["edge_attr"],
        inputs["w1"], inputs["q1"], inputs["k1"], inputs["le1"], inputs["e1"], inputs["b1"],
        inputs["w2"], inputs["q2"], inputs["k2"], inputs["le2"], inputs["e2"], inputs["b2"],
    ).astype(np.float32)
